# revision 1
# baseline (speedup 1.0000x reference)
"""Trainium2 Bass kernel for AttentiveNonLocalBlock2D.

Pipeline (per core, SPMD over 8 NeuronCores, sequence-parallel over N=H*W):
  Phase A (replicated): 3x stride-2 conv gating unit (fp16 PE); conv3 emits
    its output pre-transposed (y3T [144, 64]); the bilinear x8 upsample runs
    on the PE as y3T^T @ (Uv kron Uh) against a constant fp16 [144, 9216]
    interpolation matrix, streamed in 512-col chunks straight into sigmoid
    (ACT reads PSUM) -> x_gated chunk -> phi projection and G^T tiles.
  Pass 1: score tiles fT[m_tile, n_chunk] = phi_tile^T theta_chunk (fp16 PE),
    exp(f - 7.5) via ACT. Every third tile ACT writes the fp8 cache directly
    and its f32 accumulator yields Z[m]; other tiles go through a cycling
    fp16 buffer, a DVE tensor_scalar (4x mode, dead store + accum) for Z,
    and a Pool-engine fp16->fp8e4 conversion. Z needs an AllReduce over n.
  Z is AllReduced in 5 segments; per segment G is scaled by 64/Z (Pool) and
    split into fp8 high+residual parts (DVE).  Pass 2 runs both parts as
    fp8 DoubleRow matmuls (2 m-tiles per instruction, 0.5 cy/col: out[64,n]
    += (64 G[:,m]/Z[m]) * exp8[m,n]) into a cycling PSUM bank, interleaved
    between later pass-1 tiles so only the last small segment trails the
    final exp; the read-out applies the 1/64 and adds the x_gated residual.
  Host concatenates the per-core n-chunks.
"""

import sys

if "/opt/trn_rl_repo" not in sys.path:
    sys.path.insert(0, "/opt/trn_rl_repo")

import numpy as np

NCORES = 8
C, CI, H, W = 64, 32, 96, 96
N = H * W            # 9216
CH = N // NCORES     # 1152 pixels per core
MT = N // 128        # 72 m-tiles
NA = 24              # m-tiles whose fp8 cache lives in the early SBUF pool
SEGS = ((0, 24), (24, 42), (42, 58), (58, 68), (68, 72))  # Z-AR segments
SEG_MARGIN = 6       # tiles between AR issue and first pass-2 emit
                     # (must cover the ~7us z DMA chain + scale latency, or
                     # the segment's pass-2 matmuls park at the PE queue
                     # head and starve ACT)
SUBS = ((0, 512), (512, 512), (1024, 128))  # n-chunk psum subtiles
EXP_BIAS = -7.5      # keeps exp(f+bias) <= ~110 so the fp8e4 cache never
                     # saturates (240 max IEEE e4m3); softmax-invariant
GSCALE = 64.0        # G is scaled by GSCALE/Z before the fp8 split so the
                     # residual part clears the e4m3 subnormal floor; the
                     # final read-out divides it back out
NEARLY = 18          # pass-1 tiles interleaved into the A2 chunk loop
QD = 12              # depth of the cycling fp16 exp buffer (tiles)

_compiled = {}


def _build(single=False):
    import concourse.bacc as bacc
    import concourse.bass as bass
    import concourse.mybir as mybir
    import concourse.tile as tile
    from concourse import masks

    f16 = mybir.dt.float16
    f32 = mybir.dt.float32
    f8 = mybir.dt.float8e4
    DR = mybir.MatmulPerfMode.DoubleRow
    AF = mybir.ActivationFunctionType
    ALU = mybir.AluOpType

    nc = bacc.Bacc("TRN2", target_bir_lowering=False, debug=False,
                   num_devices=1 if single else NCORES)

    xpad_io = nc.dram_tensor("xpad", [C, 98, 98], f16, kind="ExternalInput")
    x32_io = nc.dram_tensor("x32", [C, N], f32, kind="ExternalInput")
    w1_io = nc.dram_tensor("w1", [C, 9 * C], f16, kind="ExternalInput")
    w2_io = nc.dram_tensor("w2", [C, 9 * C], f16, kind="ExternalInput")
    w3_io = nc.dram_tensor("w3", [C, 9 * C], f16, kind="ExternalInput")
    krA_io = nc.dram_tensor("krA", [72, N], f16, kind="ExternalInput")
    krB_io = nc.dram_tensor("krB", [72, N], f16, kind="ExternalInput")
    twT_io = nc.dram_tensor("twT", [C, CI], f16, kind="ExternalInput")
    pwT_io = nc.dram_tensor("pwT", [C, CI], f16, kind="ExternalInput")
    gw_io = nc.dram_tensor("gw", [CI, C], f32, kind="ExternalInput")
    WwT_io = nc.dram_tensor("WwT", [CI, C], f32, kind="ExternalInput")
    xch_io = nc.dram_tensor("xch", [C, CH], f32, kind="ExternalInput")
    krC_io = nc.dram_tensor("krC", [144, CH], f16, kind="ExternalInput")
    out_io = nc.dram_tensor("out", [C, CH], f32, kind="ExternalOutput")

    with tile.TileContext(nc) as tc:
        with tc.tile_pool(name="persist", bufs=1) as pp, \
             tc.tile_pool(name="dram", bufs=1, space="DRAM") as dp:
            zsum = pp.tile([128, MT], f32)
            nb5 = pp.tile([128, 1], f32)
            nc.gpsimd.memset(nb5[:], EXP_BIAS)
            zin = [dp.tile([128, j1 - j0], f32, name=f"zin{k}")
                   for k, (j0, j1) in enumerate(SEGS)]
            zout = [dp.tile([128, j1 - j0], f32, addr_space="Shared",
                            name=f"zout{k}")
                    for k, (j0, j1) in enumerate(SEGS)]

            # cross-phase SBUF tensors
            with tc.tile_pool(name="hand", bufs=1) as hp, \
                 tc.tile_pool(name="scA", bufs=1) as scpA:
                phi16 = hp.tile([CI, N], f16)
                th16 = hp.tile([CI, CH], f16)
                G16 = hp.tile([128, MT * C], f16)
                G3 = G16[:].rearrange("p (j c) -> p j c", c=C)
                G8 = hp.tile([128, MT * C], f8)
                G83 = G8[:].rearrange("p (j c) -> p j c", c=C)
                R8 = hp.tile([128, MT * C], f8)
                R83 = R8[:].rearrange("p (j c) -> p j c", c=C)
                xgc16 = hp.tile([C, CH], f16)
                outsb = hp.tile([C, CH], f32)
                zscr = hp.tile([128, CH], f16)
                s16cyc = hp.tile([128, QD * CH], f16)
                s8A = scpA.tile([128, NA * CH], f8)
                s8A3 = s8A[:].rearrange("p (j n) -> p j n", n=CH)

                def s8_pair(j, o0, w):
                    # fp8 cache slice [128, 2, w] for m-tiles (j, j+1)
                    if j < NA:
                        return s8A3[:, j:j + 2, o0:o0 + w]
                    r = j - NA
                    return s8B3[:, r:r + 2, o0:o0 + w]

                def s8_sl(j):
                    if j < NA:
                        return s8A[:, j * CH:(j + 1) * CH]
                    r = j - NA
                    return s8B[:, r * CH:(r + 1) * CH]

                def pass1_tile(j, _p1ps):
                    fps = _p1ps.tile([128, CH], f32, tag="fps", name="fps")
                    for o0, w in SUBS:
                        nc.tensor.matmul(fps[:, o0:o0 + w],
                                         phi16[:, j * 128:(j + 1) * 128],
                                         th16[:, o0:o0 + w],
                                         start=True, stop=True)
                    if j % 3 == 0:
                        # ACT writes the fp8 cache directly; its f32
                        # accumulator gives Z (+187ns) - keeps Pool/DVE free
                        nc.scalar.activation(s8_sl(j), fps[:], AF.Exp,
                                             bias=nb5[:], scale=1.0,
                                             accum_out=zsum[:, j:j + 1])
                        return
                    q = j % QD
                    cyc = s16cyc[:, q * CH:(q + 1) * CH]
                    nc.scalar.activation(cyc, fps[:], AF.Exp,
                                         bias=nb5[:], scale=1.0)
                    # free-axis sum: fp16 dead store runs at 4x on DVE, the
                    # live f32 accum_out gives Z[m] (360ns vs 1260ns reduce)
                    nc.vector.tensor_scalar(
                        zscr[:], cyc, 1.0, 0.0, op0=ALU.mult, op1=ALU.add,
                        accum_out=zsum[:, j:j + 1])
                    # persist as fp8 on the otherwise-idle Pool engine
                    nc.gpsimd.tensor_copy(s8_sl(j), cyc)

                # ==================== PHASE A ====================
                with tc.tile_pool(name="mid", bufs=1) as pm:
                    # small constants first: they must not share addresses
                    # with (or queue behind) the conv-phase pool, else the
                    # own-chunk critical section drains on the conv tail
                    twT16 = pm.tile([C, CI], f16)
                    nc.sync.dma_start(twT16[:], twT_io[:])
                    pwT16 = pm.tile([C, CI], f16)
                    nc.sync.dma_start(pwT16[:], pwT_io[:])
                    gwsb = pm.tile([CI, C], f32)
                    nc.sync.dma_start(gwsb[:], gw_io[:])
                    WwTsb = pm.tile([CI, C], f32)
                    nc.sync.dma_start(WwTsb[:], WwT_io[:])
                    xchsb = pm.tile([C, CH], f32)
                    nc.sync.dma_start(xchsb[:], xch_io[:])
                    krCa = pm.tile([72, CH], f16)
                    nc.sync.dma_start(krCa[:], krC_io[0:72, :])
                    krCb = pm.tile([72, CH], f16)
                    nc.sync.dma_start(krCb[:], krC_io[72:144, :])
                    ka16 = pm.tile([72, N], f16)
                    kb16 = pm.tile([72, N], f16)
                    y3Ta = pm.tile([72, C], f16)
                    y3Tb = pm.tile([72, C], f16)
                    xg16 = pm.tile([C, N], f16)

                    # --- A1: convs (conv3 transposed) ---
                    with tc.tile_pool(name="pa1", bufs=1) as pa, \
                         tc.tile_pool(name="paps1", bufs=2, space="PSUM") as paps:
                        w1sb = pa.tile([C, 9 * C], f16)
                        nc.sync.dma_start(w1sb[:], w1_io[:])
                        w2sb = pa.tile([C, 9 * C], f16)
                        nc.sync.dma_start(w2sb[:], w2_io[:])
                        w3sb = pa.tile([C, 9 * C], f16)
                        nc.sync.dma_start(w3sb[:], w3_io[:])
                        xpad = pa.tile([C, 98, 98], f16)
                        for b in range(4):
                            r0, r1 = 26 * b, min(26 * b + 26, 98)
                            nc.sync.dma_start(xpad[:, r0:r1, :],
                                              xpad_io[:, r0:r1, :])
                        # kron matrices load behind xpad (needed only after
                        # conv3, ~13us in) so they don't delay conv1
                        for b in range(4):
                            sl = slice(b * 2304, (b + 1) * 2304)
                            nc.sync.dma_start(ka16[:, sl], krA_io[:, sl])
                            nc.sync.dma_start(kb16[:, sl], krB_io[:, sl])

                        # conv1: 96x96 -> 48x48, stride 2, pad 1, lrelu(0.2)
                        y1p = pa.tile([C, 50, 50], f16)
                        nc.gpsimd.memset(y1p[:], 0.0)
                        for g in range(6):
                            ps1 = paps.tile([C, 8, 48], f32, tag="cv", name="ps1")
                            for t in range(9):
                                dy, dx = t // 3, t % 3
                                nc.tensor.matmul(
                                    ps1[:], w1sb[:, t * C:(t + 1) * C],
                                    xpad[:, 16 * g + dy: 16 * g + dy + 16: 2,
                                         dx: dx + 96: 2],
                                    start=(t == 0), stop=(t == 8))
                            # lrelu(x) = 0.6*x + 0.4*|x|
                            ab1 = pa.tile([C, 8 * 48], f32, tag="ab1", name="ab1",
                                          bufs=2)
                            nc.scalar.activation(ab1[:], ps1[:], AF.Abs, scale=0.4)
                            nc.vector.scalar_tensor_tensor(
                                y1p[:, 1 + 8 * g: 9 + 8 * g, 1:49], ps1[:], 0.6,
                                ab1[:], op0=ALU.mult, op1=ALU.add)

                        # conv2: 48x48 -> 24x24
                        y2p = pa.tile([C, 26, 26], f16)
                        nc.gpsimd.memset(y2p[:], 0.0)
                        for g in range(2):
                            ps2 = paps.tile([C, 12, 24], f32, tag="cv", name="ps2")
                            for t in range(9):
                                dy, dx = t // 3, t % 3
                                nc.tensor.matmul(
                                    ps2[:], w2sb[:, t * C:(t + 1) * C],
                                    y1p[:, 24 * g + dy: 24 * g + dy + 24: 2,
                                        dx: dx + 48: 2],
                                    start=(t == 0), stop=(t == 8))
                            ab2 = pa.tile([C, 12 * 24], f32, tag="ab2", name="ab2",
                                          bufs=2)
                            nc.scalar.activation(ab2[:], ps2[:], AF.Abs, scale=0.4)
                            nc.vector.scalar_tensor_tensor(
                                y2p[:, 1 + 12 * g: 13 + 12 * g, 1:25], ps2[:], 0.6,
                                ab2[:], op0=ALU.mult, op1=ALU.add)

                        # conv3: 24x24 -> 12x12 (no activation), then PE
                        # transpose into y3T[(row, col), c] halves for the
                        # kron-upsample matmul
                        ps3 = paps.tile([C, 12, 12], f32, tag="cv", name="ps3")
                        for t in range(9):
                            dy, dx = t // 3, t % 3
                            nc.tensor.matmul(
                                ps3[:], w3sb[:, t * C:(t + 1) * C],
                                y2p[:, dy: dy + 24: 2, dx: dx + 24: 2],
                                start=(t == 0), stop=(t == 8))
                        y3f = pa.tile([C, 144], f16)
                        nc.vector.tensor_copy(y3f[:], ps3[:])
                        ident = pa.tile([C, C], f16)
                        masks.make_identity(nc, ident[:])
                        for hh, y3t in ((0, y3Ta), (1, y3Tb)):
                            pst = paps.tile([72, C], f16, tag="c3", name="pst")
                            nc.tensor.transpose(
                                pst[:], y3f[:, 72 * hh:72 * (hh + 1)],
                                ident[:])
                            nc.vector.tensor_copy(y3t[:], pst[:])

                    # --- A2: upsample-as-matmul, gate, x_gated, projections
                    with tc.tile_pool(name="pa2", bufs=1) as pa, \
                         tc.tile_pool(name="paps2", bufs=1, space="PSUM") as paps:
                        # E^T = gw^T WwT [C, C] (fp16 for 1 cy/col G^T mms)
                        eps = paps.tile([C, 512], f32, tag="prj", name="eps",
                                        bufs=5)
                        nc.tensor.matmul(eps[:, 0:C], gwsb[:], WwTsb[:],
                                         start=True, stop=True)
                        ET16 = pa.tile([C, C], f16)
                        nc.vector.tensor_copy(ET16[:], eps[:, 0:C])

                        # early own-chunk path: dynamic kron columns ->
                        # sigmoid -> x_gated chunk -> theta; unblocks pass 1
                        # own-chunk kron from the per-core krC input: no
                        # dynamic offsets, no tile_critical drain
                        gtc = pa.tile([C, CH], f32)
                        for o0, w in SUBS:
                            kps = paps.tile([C, 512], f32, tag="prj",
                                            name="kps", bufs=5)
                            nc.tensor.matmul(
                                kps[:, 0:w], y3Ta[:],
                                krCa[:, o0:o0 + w],
                                start=True, stop=False)
                            nc.tensor.matmul(
                                kps[:, 0:w], y3Tb[:],
                                krCb[:, o0:o0 + w],
                                start=False, stop=True)
                            nc.scalar.activation(gtc[:, o0:o0 + w],
                                                 kps[:, 0:w], AF.Sigmoid)
                        nc.vector.tensor_mul(xgc16[:], gtc[:], xchsb[:])

                        # theta chunk [CI, CH] fp16 matmuls
                        for o0, w in SUBS:
                            tps = paps.tile([C, 512], f32, tag="prj",
                                            name="tps", bufs=5)
                            nc.tensor.matmul(tps[0:CI, 0:w], twT16[:],
                                             xgc16[:, o0:o0 + w],
                                             start=True, stop=True)
                            nc.vector.tensor_copy(th16[:, o0:o0 + w],
                                                  tps[0:CI, 0:w])

                        # software-pipelined 18-chunk loop, 2-chunk skew:
                        # chunk i-1's first xg consumers and chunk i-2's
                        # tail are emitted around kron(i) so at most 3 PE
                        # instructions ever park in the 4-deep wait queue
                        gps_t = {}

                        def xg_a(i, paps=paps):
                            # phi projection only: its stationary is a
                            # constant, so nothing Ldweights-stalls on the
                            # just-computed xg16
                            sl = slice(i * 512, (i + 1) * 512)
                            pps = paps.tile([C, 512], f32, tag="prj",
                                            name="pps", bufs=5)
                            nc.tensor.matmul(pps[0:CI, :], pwT16[:],
                                             xg16[:, sl],
                                             start=True, stop=True)
                            if i % 2 == 0:
                                nc.scalar.copy(phi16[:, sl], pps[0:CI, :])
                            else:
                                nc.vector.tensor_copy(phi16[:, sl],
                                                      pps[0:CI, :])

                        gps_t = {}

                        def xg_b1(i, paps=paps):
                            # G^T tiles use xg16 slices as the STATIONARY;
                            # a not-yet-written stationary blocks the PE SEQ
                            # in Ldweights, so run these 2 chunks late; split
                            # 2+2 so at most 3 PE instructions ever park
                            gps = paps.tile([128, 4 * C], f32, tag="gps",
                                            name="gps", bufs=2)
                            gps_t[i] = gps
                            for u in range(2):
                                j = 4 * i + u
                                nc.tensor.matmul(gps[:, u * C:(u + 1) * C],
                                                 xg16[:, j * 128:(j + 1) * 128],
                                                 ET16[:], start=True,
                                                 stop=True)

                        def xg_b2(i):
                            gps = gps_t.pop(i)
                            for u in range(2, 4):
                                j = 4 * i + u
                                nc.tensor.matmul(gps[:, u * C:(u + 1) * C],
                                                 xg16[:, j * 128:(j + 1) * 128],
                                                 ET16[:], start=True,
                                                 stop=True)
                            nc.vector.tensor_copy(
                                G16[:, 4 * i * C:(4 * i + 4) * C], gps[:])

                        for i in range(21):
                            if i < 18:
                                sl = slice(i * 512, (i + 1) * 512)
                                kps = paps.tile([C, 512], f32, tag="prj",
                                                name="kps", bufs=5)
                                nc.tensor.matmul(kps[:], y3Ta[:], ka16[:, sl],
                                                 start=True, stop=False)
                                nc.tensor.matmul(kps[:], y3Tb[:], kb16[:, sl],
                                                 start=False, stop=True)
                                gt = pa.tile([C, 512], f32, tag="gt",
                                             name="gt", bufs=6)
                                nc.scalar.activation(gt[:], kps[:],
                                                     AF.Sigmoid)
                                xc = pa.tile([C, 512], f32, tag="xc",
                                             name="xc", bufs=6)
                                nc.sync.dma_start(xc[:], x32_io[:, sl])
                                meng = nc.vector if i % 2 == 0 else nc.gpsimd
                                meng.tensor_mul(xg16[:, sl], gt[:], xc[:])
                            if 1 <= i <= 18:
                                xg_a(i - 1)
                            if 2 <= i <= 19:
                                xg_b1(i - 2)
                            if 3 <= i <= 20:
                                xg_b2(i - 3)

                # ====== PASS 1 with seg-interleaved fp8 PASS 2 + ARs ======
                with tc.tile_pool(name="scache", bufs=1) as scp, \
                     tc.tile_pool(name="p1ps", bufs=2, space="PSUM") as p1ps:
                    s8B = scp.tile([128, (MT - NA) * CH], f8)
                    s8B3 = s8B[:].rearrange("p (j n) -> p j n", n=CH)

                    def allreduce(k):
                        j0, j1 = SEGS[k]
                        nc.sync.dma_start(zin[k][:], zsum[:, j0:j1])
                        if single:
                            nc.sync.dma_start(zout[k][:], zin[k][:])
                        else:
                            nc.gpsimd.collective_compute(
                                "AllReduce", ALU.add,
                                replica_groups=[list(range(NCORES))],
                                ins=[zin[k].opt()], outs=[zout[k].opt()])

                    def scale_G(k, p2):
                        j0, j1 = SEGS[k]
                        ln = j1 - j0
                        zf = p2.tile([128, 24], f32, tag="zf", name="zf",
                                     bufs=2)
                        nc.sync.dma_start(zf[:, 0:ln], zout[k][:])
                        # fold the fp8 pre-scale into the reciprocal input:
                        # 1/(Z/GSCALE) = GSCALE/Z
                        nc.vector.tensor_scalar(zf[:, 0:ln], zf[:, 0:ln],
                                                1.0 / GSCALE, None,
                                                op0=ALU.mult)
                        rz = p2.tile([128, 24], f32, tag="rz", name="rz",
                                     bufs=2)
                        nc.vector.reciprocal(rz[:, 0:ln], zf[:, 0:ln])
                        rzb = rz[:, 0:ln].unsqueeze(-1).to_broadcast(
                            (128, ln, C))
                        # on DVE: Pool runs at its fp8-conversion limit here,
                        # and a Pool backlog stalls ACT via the cyc slots
                        nc.vector.tensor_mul(G3[:, j0:j1, :],
                                             G3[:, j0:j1, :], rzb)
                        # split scaled G into fp8 high + fp8 residual parts
                        nc.vector.tensor_copy(G83[:, j0:j1, :],
                                              G3[:, j0:j1, :])
                        rt = p2.tile([128, 24 * C], f16, tag="rt", name="rt",
                                     bufs=2)
                        rt3 = rt[:].rearrange("p (j c) -> p j c", c=C)
                        nc.vector.tensor_sub(rt3[:, 0:ln, :],
                                             G3[:, j0:j1, :],
                                             G83[:, j0:j1, :])
                        nc.vector.tensor_copy(R83[:, j0:j1, :],
                                              rt3[:, 0:ln, :])

                    with tc.tile_pool(name="p2", bufs=1) as p2, \
                         tc.tile_pool(name="p2ps", bufs=1, space="PSUM") as p2ps:
                        # pass-2 work units: per segment, per n-chunk, a
                        # cycling PSUM tile accumulates that segment's
                        # m-tile pairs (fp8 DoubleRow = 2 m-tiles per
                        # matmul at 0.5 cy/col; DR requires out partition
                        # base 0, so no row packing); G-high and G-residual
                        # passes accumulate into the same bank
                        units = []  # (k, ci, u) in emission order
                        for k in range(len(SEGS)):
                            j0, j1 = SEGS[k]
                            for ci in range(len(SUBS)):
                                for u in range((j1 - j0) // 2):
                                    units.append((k, ci, u))
                        cur_ps = {}
                        emitted = 0

                        def emit_unit():
                            nonlocal emitted
                            k, ci, u = units[emitted]
                            j0, j1 = SEGS[k]
                            o0, w = SUBS[ci]
                            npr = (j1 - j0) // 2
                            if u == 0:
                                cur_ps[ci] = p2ps.tile(
                                    [C, 512], f32, tag="yps",
                                    name="yps", bufs=2)
                            yps = cur_ps[ci]
                            jj = j0 + 2 * u
                            first, last = u == 0, u == npr - 1
                            nc.tensor.matmul(
                                yps[:, 0:w], G83[:, jj:jj + 2, :],
                                s8_pair(jj, o0, w),
                                start=first, stop=False, perf_mode=DR,
                                skip_group_check=True)
                            nc.tensor.matmul(
                                yps[:, 0:w], R83[:, jj:jj + 2, :],
                                s8_pair(jj, o0, w),
                                start=False, stop=last, perf_mode=DR,
                                skip_group_check=True)
                            if last:
                                osl = outsb[:, o0:o0 + w]
                                if k == 0:
                                    nc.vector.tensor_copy(osl, yps[:, 0:w])
                                else:
                                    nc.vector.tensor_add(osl, osl,
                                                         yps[:, 0:w])
                                if k == len(SEGS) - 1:
                                    # undo the GSCALE pre-scale and add the
                                    # x_gated residual
                                    nc.vector.scalar_tensor_tensor(
                                        osl, osl, 1.0 / GSCALE,
                                        xgc16[:, o0:o0 + w],
                                        op0=ALU.mult, op1=ALU.add)
                                    nc.sync.dma_start(
                                        out_io[:, o0:o0 + w], osl)
                            emitted += 1

                        # units of segment k become eligible once scale_G(k)
                        # has been issued + SEG_MARGIN tiles
                        seg_done = [s[1] for s in SEGS]
                        seg_units = [sum(1 for x in units if x[0] <= k)
                                     for k in range(len(SEGS))]
                        avail = [0]

                        def pump(j):
                            for k in range(len(SEGS)):
                                if j == seg_done[k] - 1:
                                    allreduce(k)
                                    scale_G(k, p2)
                                if j == seg_done[k] - 1 + SEG_MARGIN:
                                    avail[0] = seg_units[k]
                            # pace pass-2 emission so PE work per tile stays
                            # under the ACT exp cadence AND units spread
                            # evenly (a PE idle gap drops the pstate to half
                            # speed for 3us, so bursts after gaps stall ACT)
                            budget = 2
                            while emitted < avail[0] and budget > 0:
                                emit_unit()
                                budget -= 1

                        for j in range(MT):
                            pass1_tile(j, p1ps)
                            pump(j)
                        while emitted < len(units):
                            emit_unit()

    nc.compile()
    return nc


def get_program():
    if "nc" not in _compiled:
        _compiled["nc"] = _build()
    return _compiled["nc"]


def _bilinear_kron():
    """K[(k,j), (R,Cc)] = uv[R,k]*uv[Cc,j] for x8 bilinear upsample 12->96
    (align_corners=False, edge-clamped), split into two 72-row halves."""
    uv = np.zeros((96, 12), np.float64)
    for R in range(96):
        t = (R + 0.5) / 8.0 - 0.5
        k0 = int(np.floor(t))
        fr = t - k0
        for k, wt in ((k0, 1.0 - fr), (k0 + 1, fr)):
            kc = min(max(k, 0), 11)
            uv[R, kc] += wt
    K = np.einsum("Rk,Cj->kjRC", uv, uv).reshape(144, 9216)
    K = np.ascontiguousarray(K).astype(np.float16)
    return K[0:72], K[72:144]


def make_in_maps(inputs):
    f16 = np.float16
    x = np.asarray(inputs["x"], np.float32).reshape(C, H, W)
    xflat = np.ascontiguousarray(x.reshape(C, N))
    xpad = np.zeros((C, 98, 98), f16)
    xpad[:, 1:97, 1:97] = x.astype(f16)
    krA, krB = _bilinear_kron()

    def conv_w(w):
        # [o, i, dy, dx] -> [i, (dy dx), o]
        return np.ascontiguousarray(
            np.asarray(w, np.float32).transpose(1, 2, 3, 0).reshape(C, 9 * C)
        ).astype(f16)

    base = {
        "xpad": xpad,
        "x32": xflat,
        "w1": conv_w(inputs["d1_w"]),
        "w2": conv_w(inputs["d2_w"]),
        "w3": conv_w(inputs["d3_w"]),
        "krA": krA,
        "krB": krB,
        "twT": np.ascontiguousarray(
            np.asarray(inputs["th_w"], np.float32)[:, :, 0, 0].T).astype(f16),
        "pwT": np.ascontiguousarray(
            np.asarray(inputs["ph_w"], np.float32)[:, :, 0, 0].T).astype(f16),
        "gw": np.ascontiguousarray(
            np.asarray(inputs["g_w"], np.float32)[:, :, 0, 0]),
        "WwT": np.ascontiguousarray(
            np.asarray(inputs["W_w"], np.float32)[:, :, 0, 0].T),
    }
    krF = np.concatenate([krA, krB], axis=0)
    in_maps = []
    for k in range(NCORES):
        m = dict(base)
        m["xch"] = np.ascontiguousarray(xflat[:, k * CH:(k + 1) * CH])
        m["krC"] = np.ascontiguousarray(krF[:, k * CH:(k + 1) * CH])
        in_maps.append(m)
    return in_maps


def kernel(**inputs):
    from concourse import bass_utils

    nc = get_program()
    in_maps = make_in_maps(inputs)
    res = bass_utils.run_bass_kernel_spmd(nc, in_maps,
                                          core_ids=list(range(NCORES)))
    out = np.concatenate([res.results[k]["out"] for k in range(NCORES)], axis=1)
    return out.reshape(1, C, H, W).astype(np.float32)



# revision 70
# speedup vs baseline: 1.2758x; 1.2758x over previous
"""Trainium2 Bass kernel for AttentiveNonLocalBlock2D (v3, AllGather design).

Sequence-parallel over N=H*W across 8 cores, with the tensor-parallel scheme
from the sharding hint: each core computes the gate + projections ONLY for its
own 1152-pixel chunk, then phi [32,1152] and G^T [128,9*64] are AllGathered
(SBUF-space collectives) to form the full phi [32,9216] / G [128,72*64] every
core needs for its slice of the attention.

Per core:
  Phase A: 3x stride-2 conv gating unit (fp16 PE, lrelu as DVE max(0.2x,x));
    conv3 emits pre-transposed y3T halves; the bilinear-upsample columns for
    the OWN chunk only come from the per-core krC input (y3T^T @ krC), into
    sigmoid -> x_gated chunk -> theta/phi/G^T projections -> AllGather.
  Pass 1: score tiles fT[m_tile, n_chunk] = phi_tile^T theta_chunk (fp16 PE)
    into a 2-half PSUM ring; ACT exp(f - 7.5) writes the fp8e4 cache
    directly.  Z[m] (softmax denominator partial) comes from the ACT f32
    accumulator (every 3rd tile), else a DVE/Pool dead-store tensor_scalar
    accumulation over the fp8 cache.  Z is AllReduced (SBUF space) in 6
    segments.
  Pass 2: per segment G is scaled by 64/Z and split into fp8 high+residual;
    fp8 DoubleRow matmuls (2 m-tiles/instr, 0.5 cy/col) accumulate ALL
    segments into persistent PSUM banks, interleaved between later pass-1
    tiles; a single final read-out applies 1/64 and adds the gated residual.
  Host concatenates the per-core n-chunks.
"""

import sys

if "/opt/trn_rl_repo" not in sys.path:
    sys.path.insert(0, "/opt/trn_rl_repo")

import numpy as np

NCORES = 8
C, CI, H, W = 64, 32, 96, 96
N = H * W            # 9216
CH = N // NCORES     # 1152 pixels per core
MT = N // 128        # 72 m-tiles of 128
TPC = MT // NCORES   # 9 own m-tiles per core
EXP_BIAS = -2.5      # keeps exp(f+bias) <= ~16k < 57344 (e5m2 max) while
                     # minimizing subnormal flushing of tiny softmax terms
GSCALE = 64.0 * float(np.exp(-2.5 + 7.5))
                     # pre-scale so G*GSCALE/Z clears the e4m3 subnormal
                     # floor; tracks EXP_BIAS (Z scales with exp(bias))
SEGS = ((0, 22), (22, 40), (40, 54), (54, 64), (64, 70), (70, 72))
EIW = 1536           # exp-instruction width (cols); 54 instrs over 72 tiles
EI = MT * CH // EIW
MARGINS = (6, 5, 5, 4, 99, 99)  # exp-instrs between AR issue and pass-2
                                # emit; last two segs drain after the loop
BUDGET = (3, 4)      # pass-2 units per exp instr (early, late)
RESID = True         # add an fp8 residual pass for G (extra accuracy)
# n-chunk subtiles for the two PSUM ring halves (bank-boundary aligned)
SUBS0 = ((0, 512), (512, 512), (1024, 128))
SUBS1 = ((0, 384), (384, 512), (896, 256))
YSUBS = ((0, 512), (512, 512), (1024, 128))  # pass-2 output subtiles

_compiled = {}


def _zmode(j):
    """Z accumulation engine per tile: DVE only (the dead-store
    tensor_scalar opcode does not exist on GPSIMD, and ACT's accumulator
    cannot be used because exp instructions span m-tile boundaries)."""
    return "dve"


def _seg_of(j):
    for k, (j0, j1) in enumerate(SEGS):
        if j0 <= j < j1:
            return k, j0
    raise ValueError(j)


def _build(single=False):
    import concourse.bacc as bacc
    import concourse.bass as bass
    import concourse.mybir as mybir
    import concourse.tile as tile
    from concourse import masks

    f16 = mybir.dt.float16
    f32 = mybir.dt.float32
    f8 = mybir.dt.float8e4
    f8w = mybir.dt.float8e5   # exp cache: wide range so tiny softmax terms
                              # aren't flushed (Z would lose ~10% of its mass)
    DR = mybir.MatmulPerfMode.DoubleRow
    AF = mybir.ActivationFunctionType
    ALU = mybir.AluOpType

    nc = bacc.Bacc("TRN2", target_bir_lowering=False, debug=False,
                   num_devices=1 if single else NCORES)

    xpad_io = nc.dram_tensor("xpad", [C, 98, 98], f16, kind="ExternalInput")
    w1_io = nc.dram_tensor("w1", [C, 9 * C], f16, kind="ExternalInput")
    w2_io = nc.dram_tensor("w2", [C, 9 * C], f16, kind="ExternalInput")
    w3_io = nc.dram_tensor("w3", [C, 9 * C], f16, kind="ExternalInput")
    twT_io = nc.dram_tensor("twT", [C, CI], f16, kind="ExternalInput")
    pwT_io = nc.dram_tensor("pwT", [C, CI], f16, kind="ExternalInput")
    gw_io = nc.dram_tensor("gw", [CI, C], f32, kind="ExternalInput")
    WwT_io = nc.dram_tensor("WwT", [CI, C], f32, kind="ExternalInput")
    xch_io = nc.dram_tensor("xch", [C, CH], f16, kind="ExternalInput")
    krC_io = nc.dram_tensor("krC", [144, CH], f16, kind="ExternalInput")
    out_io = nc.dram_tensor("out", [C, CH], f32, kind="ExternalOutput")

    groups = [list(range(NCORES))]

    with tile.TileContext(nc) as tc:
        with tc.tile_pool(name="persist", bufs=1) as pp, \
             tc.tile_pool(name="dram", bufs=1, space="DRAM") as dp:
            # per-segment Z tiles so the AR DMA reads never alias later writes
            zsumk = [pp.tile([128, j1 - j0], f32, name=f"zsum{k}")
                     for k, (j0, j1) in enumerate(SEGS)]
            zredk = [pp.tile([128, j1 - j0], f32, name=f"zred{k}")
                     for k, (j0, j1) in enumerate(SEGS)]
            nb5 = pp.tile([128, 1], f32)
            nc.gpsimd.memset(nb5[:], EXP_BIAS)
            zin = [dp.tile([128, j1 - j0], f32, name=f"zin{k}")
                   for k, (j0, j1) in enumerate(SEGS)]
            zout = [dp.tile([128, j1 - j0], f32, addr_space="Shared",
                            name=f"zout{k}")
                    for k, (j0, j1) in enumerate(SEGS)]
            phin = dp.tile([CI, CH], f16, name="phin")
            phout = dp.tile([NCORES, CI, CH], f16, addr_space="Shared",
                            name="phout")
            gin = dp.tile([128, TPC * C], f16, name="gin")
            gout = dp.tile([NCORES, 128, TPC * C], f16, addr_space="Shared",
                           name="gout")

            with tc.tile_pool(name="hand", bufs=1) as hp:
                phi16 = hp.tile([CI, N], f16)
                th16 = hp.tile([CI, CH], f16)
                G16 = hp.tile([128, MT * C], f16)
                G3 = G16[:].rearrange("p (j c) -> p j c", c=C)
                G8 = hp.tile([128, MT * C], f8)
                G83 = G8[:].rearrange("p (j c) -> p j c", c=C)
                R8 = hp.tile([128, MT * C], f8)
                R83 = R8[:].rearrange("p (j c) -> p j c", c=C)
                xgc16 = hp.tile([C, CH], f16)
                outsb = hp.tile([C, CH], f32)
                # (exp-table load is implicit before the first pass-1 exp;
                # it hides behind the phi AllGather landing wait)
                zdeadV = hp.tile([128, CH], f8w)  # dead stores for Z accum
                zdeadP = hp.tile([128, CH], f8w)  # (same dtype as the cache)
                phiown = hp.tile([CI, CH], f16)
                gown = hp.tile([128, TPC * C], f16)
                s8 = hp.tile([128, MT * CH], f8w)
                s83 = s8[:].rearrange("p (j n) -> p j n", n=CH)

                # ==================== PHASE A ====================
                # single merged pool scope: no mid-phase close barrier
                # between the convs and the gate/projection pipeline
                with tc.tile_pool(name="pa", bufs=1) as pa, \
                     tc.tile_pool(name="paps", bufs=2, space="PSUM") as paps:
                    y3Ta = pa.tile([72, C], f16)
                    y3Tb = pa.tile([72, C], f16)
                    # preload the Sigmoid table while input DMAs fly
                    tld0 = pa.tile([128, 1], f32)
                    nc.scalar.activation(tld0[:], nb5[:], AF.Sigmoid)
                    # ramp the PE p-state during the input-DMA wait so conv1
                    # runs at full speed from its first matmul (identity
                    # needs no DMA)
                    ident = pa.tile([C, C], f16)
                    masks.make_identity(nc, ident[:])
                    wmps = paps.tile([C, C], f32, tag="warm", name="wmps",
                                     bufs=1)
                    for _ in range(140):
                        nc.tensor.matmul(wmps[:], ident[:], ident[:],
                                         start=True, stop=True,
                                         skip_group_check=True)

                    # conv-critical DMAs first: HWDGE is one serial queue,
                    # and conv1 must run gapless to keep the PE p-state up
                    xpad = pa.tile([C, 98, 98], f16)
                    w1sb = pa.tile([C, 9 * C], f16)
                    nc.sync.dma_start(xpad[:, 0:50, :], xpad_io[:, 0:50, :])
                    nc.sync.dma_start(w1sb[:], w1_io[:])
                    nc.sync.dma_start(xpad[:, 50:98, :], xpad_io[:, 50:98, :])
                    w2sb = pa.tile([C, 9 * C], f16)
                    nc.sync.dma_start(w2sb[:], w2_io[:])
                    w3sb = pa.tile([C, 9 * C], f16)
                    nc.sync.dma_start(w3sb[:], w3_io[:])
                    twT16 = pa.tile([C, CI], f16)
                    nc.sync.dma_start(twT16[:], twT_io[:])
                    pwT16 = pa.tile([C, CI], f16)
                    nc.sync.dma_start(pwT16[:], pwT_io[:])
                    gwsb = pa.tile([CI, C], f32)
                    nc.sync.dma_start(gwsb[:], gw_io[:])
                    WwTsb = pa.tile([CI, C], f32)
                    nc.sync.dma_start(WwTsb[:], WwT_io[:])
                    krCa = pa.tile([72, CH], f16)
                    nc.sync.dma_start(krCa[:], krC_io[0:72, :])
                    krCb = pa.tile([72, CH], f16)
                    nc.sync.dma_start(krCb[:], krC_io[72:144, :])
                    xchsb = pa.tile([C, CH], f16)
                    nc.sync.dma_start(xchsb[:], xch_io[:])

                    # conv1: 96x96 -> 48x48, stride 2, pad 1, lrelu(0.2)
                    y1p = pa.tile([C, 50, 50], f16)
                    nc.gpsimd.memset(y1p[:], 0.0)
                    for g in range(6):
                        ps1 = paps.tile([C, 8, 48], f32, tag="cv", name="ps1")
                        for t in range(9):
                            dy, dx = t // 3, t % 3
                            nc.tensor.matmul(
                                ps1[:], w1sb[:, t * C:(t + 1) * C],
                                xpad[:, 16 * g + dy: 16 * g + dy + 16: 2,
                                     dx: dx + 96: 2],
                                start=(t == 0), stop=(t == 8))
                        # lrelu(x) = 0.6*x + 0.4*|x| (only one PSUM input
                        # allowed per DVE op; ACT is idle during the convs)
                        ab1 = pa.tile([C, 8 * 48], f32, tag="ab1", name="ab1",
                                      bufs=2)
                        nc.scalar.activation(ab1[:], ps1[:], AF.Abs,
                                             scale=0.4)
                        nc.vector.scalar_tensor_tensor(
                            y1p[:, 1 + 8 * g: 9 + 8 * g, 1:49], ps1[:], 0.6,
                            ab1[:], op0=ALU.mult, op1=ALU.add)

                    # conv2: 48x48 -> 24x24
                    y2p = pa.tile([C, 26, 26], f16)
                    nc.gpsimd.memset(y2p[:], 0.0)
                    for g in range(2):
                        ps2 = paps.tile([C, 12, 24], f32, tag="cv", name="ps2")
                        for t in range(9):
                            dy, dx = t // 3, t % 3
                            nc.tensor.matmul(
                                ps2[:], w2sb[:, t * C:(t + 1) * C],
                                y1p[:, 24 * g + dy: 24 * g + dy + 24: 2,
                                    dx: dx + 48: 2],
                                start=(t == 0), stop=(t == 8))
                        ab2 = pa.tile([C, 12 * 24], f32, tag="ab2", name="ab2",
                                      bufs=2)
                        nc.scalar.activation(ab2[:], ps2[:], AF.Abs,
                                             scale=0.4)
                        nc.vector.scalar_tensor_tensor(
                            y2p[:, 1 + 12 * g: 13 + 12 * g, 1:25], ps2[:], 0.6,
                            ab2[:], op0=ALU.mult, op1=ALU.add)

                    # conv3: 24x24 -> 12x12 (no activation), then PE
                    # transpose into y3T[(row, col), c] halves
                    ps3 = paps.tile([C, 12, 12], f32, tag="cv", name="ps3")
                    for t in range(9):
                        dy, dx = t // 3, t % 3
                        nc.tensor.matmul(
                            ps3[:], w3sb[:, t * C:(t + 1) * C],
                            y2p[:, dy: dy + 24: 2, dx: dx + 24: 2],
                            start=(t == 0), stop=(t == 8))
                    y3f = pa.tile([C, 144], f16)
                    nc.vector.tensor_copy(y3f[:], ps3[:])
                    for hh, y3t in ((0, y3Ta), (1, y3Tb)):
                        pst = paps.tile([72, C], f16, tag="cv", name="pst")
                        nc.tensor.transpose(
                            pst[:], y3f[:, 72 * hh:72 * (hh + 1)], ident[:])
                        nc.vector.tensor_copy(y3t[:], pst[:])

                    # E^T = gw^T WwT [C, C]
                    eps = paps.tile([C, 512], f32, tag="prj", name="eps",
                                    bufs=3)
                    nc.tensor.matmul(eps[:, 0:C], gwsb[:], WwTsb[:],
                                     start=True, stop=True)
                    ET16 = hp.tile([C, C], f16)
                    nc.vector.tensor_copy(ET16[:], eps[:, 0:C])

                    # gate pipeline: all krons first (kron -> sigmoid ->
                    # fp16 gate-mul per sub), then the phi chain (it feeds
                    # the AllGather = the pass-1 critical path), then theta
                    gtc = pa.tile([C, CH], f16)
                    for o0, w in SUBS0:
                        kps = paps.tile([C, 512], f32, tag="prj",
                                        name="kps", bufs=3)
                        nc.tensor.matmul(kps[:, 0:w], y3Ta[:],
                                         krCa[:, o0:o0 + w],
                                         start=True, stop=False)
                        nc.tensor.matmul(kps[:, 0:w], y3Tb[:],
                                         krCb[:, o0:o0 + w],
                                         start=False, stop=True)
                        nc.scalar.activation(gtc[:, o0:o0 + w],
                                             kps[:, 0:w], AF.Sigmoid)
                        nc.vector.tensor_mul(xgc16[:, o0:o0 + w],
                                             gtc[:, o0:o0 + w],
                                             xchsb[:, o0:o0 + w])
                    for o0, w in SUBS0:
                        pps = paps.tile([C, 512], f32, tag="prj",
                                        name="pps", bufs=3)
                        nc.tensor.matmul(pps[0:CI, 0:w], pwT16[:],
                                         xgc16[:, o0:o0 + w],
                                         start=True, stop=True)
                        nc.vector.tensor_copy(phiown[:, o0:o0 + w],
                                              pps[0:CI, 0:w])
                    for o0, w in SUBS0:
                        tps = paps.tile([C, 512], f32, tag="prj",
                                        name="tps", bufs=3)
                        nc.tensor.matmul(tps[0:CI, 0:w], twT16[:],
                                         xgc16[:, o0:o0 + w],
                                         start=True, stop=True)
                        # (GPSIMD cannot read PSUM on HW: copies on DVE)
                        nc.vector.tensor_copy(th16[:, o0:o0 + w],
                                              tps[0:CI, 0:w])

                    # own G^T tiles [128, 9*C] (AllGathered later)
                    gps = paps.tile([128, TPC * C], f32, tag="gps",
                                    name="gps", bufs=1)
                    for u in range(TPC):
                        nc.tensor.matmul(gps[:, u * C:(u + 1) * C],
                                         xgc16[:, u * 128:(u + 1) * 128],
                                         ET16[:], start=True, stop=True)
                    nc.vector.tensor_copy(gown[:], gps[:])

                # ====== PASS 1 with seg-interleaved fp8 PASS 2 + ARs ======
                with tc.tile_pool(name="p1ps", bufs=2, space="PSUM") as p1ps, \
                     tc.tile_pool(name="p2ps", bufs=2, space="PSUM") as p2ps, \
                     tc.tile_pool(name="p2", bufs=1) as p2:
                    # AllGathers emitted inside this scope so no pool-close
                    # barrier or clock alignment gates pass-1 on them.
                    # single-mode convention: ONE DRAM hop stands in for
                    # upload+collective; landing DMAs are modeled in full.
                    if single:
                        nc.sync.dma_start(phout[0, :, :], phiown[:])
                    else:
                        nc.sync.dma_start(phin[:], phiown[:])
                        nc.gpsimd.collective_compute(
                            "AllGather", ALU.bypass, replica_groups=groups,
                            ins=[phin.opt()], outs=[phout.opt()])
                    # land slice r=0 first: it unblocks pass-1 tiles 0-8
                    nc.sync.dma_start(phi16[:, 0:CH], phout[0, :, :])
                    nc.sync.dma_start(
                        phi16[:, CH:].rearrange("c (r n) -> c r n",
                                                r=NCORES - 1),
                        phout[1:, :, :].rearrange("r c n -> c r n"))

                    def emit_G_ag():
                        if single:
                            nc.sync.dma_start(gout[0, :, :], gown[:])
                        else:
                            nc.sync.dma_start(gin[:], gown[:])
                            nc.gpsimd.collective_compute(
                                "AllGather", ALU.bypass,
                                replica_groups=groups,
                                ins=[gin.opt()], outs=[gout.opt()])
                        nc.sync.dma_start(
                            G16[:].rearrange("p (r n) -> p r n", r=NCORES),
                            gout[:].rearrange("r p n -> p r n"))

                    # warm the PE through the AG landing wait with fake
                    # pass-1 tiles read from phiown (already in SBUF)
                    for _ in range(3):
                        wfps = p1ps.tile([128, EIW], f32, tag="fps",
                                         name="fps")
                        for o0 in range(0, EIW, 512):
                            nc.tensor.matmul(wfps[:, o0:o0 + 512],
                                             phiown[:, 0:128],
                                             th16[:, 0:512],
                                             start=True, stop=True)

                    def pass1_instr(i):
                        # one 1536-col exp instruction = 1.33 m-tiles; the
                        # fp8 cache is contiguous so the exp span can cross
                        # m-tile boundaries; Z is per-m-tile off the cache
                        c0 = i * EIW
                        fps = p1ps.tile([128, EIW], f32, tag="fps",
                                        name="fps")
                        edges = {0, EIW}
                        for b in range(512, EIW, 512):
                            edges.add(b)
                        jlo, jhi = c0 // CH, (c0 + EIW - 1) // CH
                        for j in range(jlo, jhi + 1):
                            if c0 < j * CH < c0 + EIW:
                                edges.add(j * CH - c0)
                        edges = sorted(edges)
                        for a, b in zip(edges[:-1], edges[1:]):
                            j = (c0 + a) // CH
                            ta = c0 + a - j * CH
                            nc.tensor.matmul(fps[:, a:b],
                                             phi16[:, j * 128:(j + 1) * 128],
                                             th16[:, ta:ta + (b - a)],
                                             start=True, stop=True)
                        nc.scalar.activation(s8[:, c0:c0 + EIW], fps[:],
                                             AF.Exp, bias=nb5[:], scale=1.0)
                        # Z for every m-tile completed by this instruction
                        for j in range(jlo, jhi + 1):
                            if (j + 1) * CH <= c0 + EIW:
                                k, j0 = _seg_of(j)
                                zcol = zsumk[k][:, j - j0:j - j0 + 1]
                                if _zmode(j) == "dve":
                                    eng, zd = nc.vector, zdeadV
                                else:
                                    eng, zd = nc.gpsimd, zdeadP
                                eng.tensor_scalar(
                                    zd[:], s83[:, j, :], 1.0, 0.0,
                                    op0=ALU.mult, op1=ALU.add,
                                    accum_out=zcol)

                    def allreduce(k):
                        # single-mode convention (as for the AllGathers):
                        # one DRAM hop stands in for upload+collective
                        if single:
                            nc.sync.dma_start(zout[k][:], zsumk[k][:])
                        else:
                            nc.sync.dma_start(zin[k][:], zsumk[k][:])
                            nc.gpsimd.collective_compute(
                                "AllReduce", ALU.add,
                                replica_groups=groups,
                                ins=[zin[k].opt()], outs=[zout[k].opt()])
                        nc.sync.dma_start(zredk[k][:], zout[k][:])

                    def scale_G(k):
                        j0, j1 = SEGS[k]
                        ln = j1 - j0
                        zf = p2.tile([128, 22], f32, tag="zf", name="zf",
                                     bufs=2)
                        # 1/(Z/GSCALE) = GSCALE/Z
                        nc.vector.tensor_scalar(zf[:, 0:ln], zredk[k][:],
                                                1.0 / GSCALE, None,
                                                op0=ALU.mult)
                        rz = p2.tile([128, 22], f32, tag="rz", name="rz",
                                     bufs=2)
                        nc.vector.reciprocal(rz[:, 0:ln], zf[:, 0:ln])
                        rzb = rz[:, 0:ln].unsqueeze(-1).to_broadcast(
                            (128, ln, C))
                        nc.vector.tensor_mul(G3[:, j0:j1, :],
                                             G3[:, j0:j1, :], rzb)
                        nc.vector.tensor_copy(G83[:, j0:j1, :],
                                              G3[:, j0:j1, :])
                        if RESID:
                            # split G into fp8 high + fp8 residual parts
                            rt = p2.tile([128, 22 * C], f16, tag="rt",
                                         name="rt", bufs=2)
                            rt3 = rt[:].rearrange("p (j c) -> p j c", c=C)
                            nc.vector.tensor_sub(rt3[:, 0:ln, :],
                                                 G3[:, j0:j1, :],
                                                 G83[:, j0:j1, :])
                            nc.vector.tensor_copy(R83[:, j0:j1, :],
                                                  rt3[:, 0:ln, :])

                    # pass-2 work units: (k, ci, u); per-segment PSUM
                    # accumulation, DVE adds across segments into outsb
                    units = []
                    for k in range(len(SEGS)):
                        j0, j1 = SEGS[k]
                        for ci in range(len(YSUBS)):
                            for u in range((j1 - j0) // 2):
                                units.append((k, ci, u))
                    emitted = 0
                    cur_ps = {}

                    def emit_unit():
                        nonlocal emitted
                        k, ci, u = units[emitted]
                        j0, j1 = SEGS[k]
                        o0, w = YSUBS[ci]
                        npr = (j1 - j0) // 2
                        jj = j0 + 2 * u
                        if u == 0:
                            cur_ps[ci] = p2ps.tile([C, 512], f32, tag="yps",
                                                   name="yps")
                        yp = cur_ps[ci]
                        nc.tensor.matmul(
                            yp[:, 0:w], G83[:, jj:jj + 2, :],
                            s83[:, jj:jj + 2, o0:o0 + w],
                            start=(u == 0), stop=(not RESID and u == npr - 1),
                            perf_mode=DR, skip_group_check=True)
                        if RESID:
                            nc.tensor.matmul(
                                yp[:, 0:w], R83[:, jj:jj + 2, :],
                                s83[:, jj:jj + 2, o0:o0 + w],
                                start=False, stop=(u == npr - 1),
                                perf_mode=DR, skip_group_check=True)
                        if u == npr - 1:
                            eng = nc.vector
                            osl = outsb[:, o0:o0 + w]
                            if k == 0:
                                eng.tensor_copy(osl, yp[:, 0:w])
                            else:
                                eng.tensor_add(osl, osl, yp[:, 0:w])
                            if k == len(SEGS) - 1:
                                # undo GSCALE pre-scale, add gated residual
                                eng.scalar_tensor_tensor(
                                    osl, osl, 1.0 / GSCALE,
                                    xgc16[:, o0:o0 + w],
                                    op0=ALU.mult, op1=ALU.add)
                                nc.sync.dma_start(out_io[:, o0:o0 + w], osl)
                        emitted += 1

                    # m-tile j's exp completes during exp-instr ei(j)
                    def ei_of(j):
                        return ((j + 1) * CH - 1) // EIW

                    seg_ei = [ei_of(s[1] - 1) for s in SEGS]
                    seg_units = [sum(1 for x in units if x[0] <= k)
                                 for k in range(len(SEGS))]
                    avail = [0]

                    def pump(i):
                        if i == 2:
                            emit_G_ag()
                        for k in range(len(SEGS)):
                            if i == seg_ei[k]:
                                allreduce(k)
                                scale_G(k)
                            if (k < len(SEGS) - 2
                                    and i == seg_ei[k] + MARGINS[k]):
                                # last 2 segs drain after the loop, behind
                                # the PE warm-up (parked units would block
                                # the warm-up and drop the p-state)
                                avail[0] = seg_units[k]
                        budget = BUDGET[0] if i < 30 else BUDGET[1]
                        while emitted < avail[0] and budget > 0:
                            emit_unit()
                            budget -= 1

                    for i in range(EI):
                        pass1_instr(i)
                        pump(i)
                    # keep the PE p-state warm through the final Z-AR wait:
                    # re-run an already-satisfied pair into a scratch bank
                    wps = p2ps.tile([C, 512], f32, tag="yps", name="wps")
                    for _ in range(45):
                        nc.tensor.matmul(wps[:], G83[:, 0:2, :],
                                         s83[:, 0:2, 0:512],
                                         start=True, stop=True, perf_mode=DR,
                                         skip_group_check=True)
                    while emitted < len(units):
                        emit_unit()

    nc.compile()
    return nc


def get_program():
    if "nc" not in _compiled:
        _compiled["nc"] = _build()
    return _compiled["nc"]


def _bilinear_kron():
    """K[(k,j), (R,Cc)] = uv[R,k]*uv[Cc,j] for x8 bilinear upsample 12->96
    (align_corners=False, edge-clamped), split into two 72-row halves."""
    uv = np.zeros((96, 12), np.float64)
    for R in range(96):
        t = (R + 0.5) / 8.0 - 0.5
        k0 = int(np.floor(t))
        fr = t - k0
        for k, wt in ((k0, 1.0 - fr), (k0 + 1, fr)):
            kc = min(max(k, 0), 11)
            uv[R, kc] += wt
    K = np.einsum("Rk,Cj->kjRC", uv, uv).reshape(144, 9216)
    return np.ascontiguousarray(K).astype(np.float16)


def make_in_maps(inputs):
    f16 = np.float16
    x = np.asarray(inputs["x"], np.float32).reshape(C, H, W)
    xflat = np.ascontiguousarray(x.reshape(C, N))
    xpad = np.zeros((C, 98, 98), f16)
    xpad[:, 1:97, 1:97] = x.astype(f16)
    krF = _bilinear_kron()

    def conv_w(w):
        # [o, i, dy, dx] -> [i, (dy dx), o]
        return np.ascontiguousarray(
            np.asarray(w, np.float32).transpose(1, 2, 3, 0).reshape(C, 9 * C)
        ).astype(f16)

    base = {
        "xpad": xpad,
        "w1": conv_w(inputs["d1_w"]),
        "w2": conv_w(inputs["d2_w"]),
        "w3": conv_w(inputs["d3_w"]),
        "twT": np.ascontiguousarray(
            np.asarray(inputs["th_w"], np.float32)[:, :, 0, 0].T).astype(f16),
        "pwT": np.ascontiguousarray(
            np.asarray(inputs["ph_w"], np.float32)[:, :, 0, 0].T).astype(f16),
        "gw": np.ascontiguousarray(
            np.asarray(inputs["g_w"], np.float32)[:, :, 0, 0]),
        "WwT": np.ascontiguousarray(
            np.asarray(inputs["W_w"], np.float32)[:, :, 0, 0].T),
    }
    in_maps = []
    for k in range(NCORES):
        m = dict(base)
        m["xch"] = np.ascontiguousarray(
            xflat[:, k * CH:(k + 1) * CH]).astype(f16)
        m["krC"] = np.ascontiguousarray(krF[:, k * CH:(k + 1) * CH])
        in_maps.append(m)
    return in_maps


def kernel(**inputs):
    from concourse import bass_utils

    nc = get_program()
    in_maps = make_in_maps(inputs)
    res = bass_utils.run_bass_kernel_spmd(nc, in_maps,
                                          core_ids=list(range(NCORES)))
    out = np.concatenate([res.results[k]["out"] for k in range(NCORES)], axis=1)
    return out.reshape(1, C, H, W).astype(np.float32)


# revision 73
# speedup vs baseline: 1.3196x; 1.0344x over previous
"""Trainium2 Bass kernel for AttentiveNonLocalBlock2D (v3, AllGather design).

Sequence-parallel over N=H*W across 8 cores, with the tensor-parallel scheme
from the sharding hint: each core computes the gate + projections ONLY for its
own 1152-pixel chunk, then phi [32,1152] and G^T [128,9*64] are AllGathered
(SBUF-space collectives) to form the full phi [32,9216] / G [128,72*64] every
core needs for its slice of the attention.

Per core:
  Phase A: 3x stride-2 conv gating unit (fp16 PE, lrelu as DVE max(0.2x,x));
    conv3 emits pre-transposed y3T halves; the bilinear-upsample columns for
    the OWN chunk only come from the per-core krC input (y3T^T @ krC), into
    sigmoid -> x_gated chunk -> theta/phi/G^T projections -> AllGather.
  Pass 1: score tiles fT[m_tile, n_chunk] = phi_tile^T theta_chunk (fp16 PE)
    into a 2-half PSUM ring; ACT exp(f - 7.5) writes the fp8e4 cache
    directly.  Z[m] (softmax denominator partial) comes from the ACT f32
    accumulator (every 3rd tile), else a DVE/Pool dead-store tensor_scalar
    accumulation over the fp8 cache.  Z is AllReduced (SBUF space) in 6
    segments.
  Pass 2: per segment G is scaled by 64/Z and split into fp8 high+residual;
    fp8 DoubleRow matmuls (2 m-tiles/instr, 0.5 cy/col) accumulate ALL
    segments into persistent PSUM banks, interleaved between later pass-1
    tiles; a single final read-out applies 1/64 and adds the gated residual.
  Host concatenates the per-core n-chunks.
"""

import sys

if "/opt/trn_rl_repo" not in sys.path:
    sys.path.insert(0, "/opt/trn_rl_repo")

import numpy as np

NCORES = 8
C, CI, H, W = 64, 32, 96, 96
N = H * W            # 9216
CH = N // NCORES     # 1152 pixels per core
MT = N // 128        # 72 m-tiles of 128
TPC = MT // NCORES   # 9 own m-tiles per core
EXP_BIAS = -2.5      # keeps exp(f+bias) <= ~16k < 57344 (e5m2 max) while
                     # minimizing subnormal flushing of tiny softmax terms
GSCALE = 64.0 * float(np.exp(-2.5 + 7.5))
                     # pre-scale so G*GSCALE/Z clears the e4m3 subnormal
                     # floor; tracks EXP_BIAS (Z scales with exp(bias))
SEGS = ((0, 22), (22, 40), (40, 54), (54, 64), (64, 70), (70, 72))
EIW = 1536           # exp-instruction width (cols); 54 instrs over 72 tiles
EI = MT * CH // EIW
MARGINS = (7, 8, 7, 6, 99, 99)  # exp-instrs between AR issue and pass-2
                                # emit; last two segs drain after the loop
BUDGET = (3, 4)      # pass-2 units per exp instr (early, late)
RESID = True         # add an fp8 residual pass for G (extra accuracy)
# n-chunk subtiles for the two PSUM ring halves (bank-boundary aligned)
SUBS0 = ((0, 512), (512, 512), (1024, 128))
SUBS1 = ((0, 384), (384, 512), (896, 256))
YSUBS = ((0, 512), (512, 512), (1024, 128))  # pass-2 output subtiles

_compiled = {}


def _zmode(j):
    """Z accumulation engine per tile: DVE only (the dead-store
    tensor_scalar opcode does not exist on GPSIMD, and ACT's accumulator
    cannot be used because exp instructions span m-tile boundaries)."""
    return "dve"


def _seg_of(j):
    for k, (j0, j1) in enumerate(SEGS):
        if j0 <= j < j1:
            return k, j0
    raise ValueError(j)


def _build(single=False):
    import concourse.bacc as bacc
    import concourse.bass as bass
    import concourse.mybir as mybir
    import concourse.tile as tile
    from concourse import masks

    f16 = mybir.dt.float16
    f32 = mybir.dt.float32
    f8 = mybir.dt.float8e4
    f8w = mybir.dt.float8e5   # exp cache: wide range so tiny softmax terms
                              # aren't flushed (Z would lose ~10% of its mass)
    DR = mybir.MatmulPerfMode.DoubleRow
    AF = mybir.ActivationFunctionType
    ALU = mybir.AluOpType

    nc = bacc.Bacc("TRN2", target_bir_lowering=False, debug=False,
                   num_devices=1 if single else NCORES)

    xpad_io = nc.dram_tensor("xpad", [C, 98, 98], f16, kind="ExternalInput")
    w1_io = nc.dram_tensor("w1", [C, 9 * C], f16, kind="ExternalInput")
    w2_io = nc.dram_tensor("w2", [C, 9 * C], f16, kind="ExternalInput")
    w3_io = nc.dram_tensor("w3", [C, 9 * C], f16, kind="ExternalInput")
    twT_io = nc.dram_tensor("twT", [C, CI], f16, kind="ExternalInput")
    pwT_io = nc.dram_tensor("pwT", [C, CI], f16, kind="ExternalInput")
    gw_io = nc.dram_tensor("gw", [CI, C], f32, kind="ExternalInput")
    WwT_io = nc.dram_tensor("WwT", [CI, C], f32, kind="ExternalInput")
    xch_io = nc.dram_tensor("xch", [C, CH], f16, kind="ExternalInput")
    krC_io = nc.dram_tensor("krC", [144, CH], f16, kind="ExternalInput")
    out_io = nc.dram_tensor("out", [C, CH], f32, kind="ExternalOutput")

    groups = [list(range(NCORES))]

    with tile.TileContext(nc) as tc:
        with tc.tile_pool(name="persist", bufs=1) as pp, \
             tc.tile_pool(name="dram", bufs=1, space="DRAM") as dp:
            # per-segment Z tiles so the AR DMA reads never alias later writes
            zsumk = [pp.tile([128, j1 - j0], f32, name=f"zsum{k}")
                     for k, (j0, j1) in enumerate(SEGS)]
            zredk = [pp.tile([128, j1 - j0], f32, name=f"zred{k}")
                     for k, (j0, j1) in enumerate(SEGS)]
            nb5 = pp.tile([128, 1], f32)
            nc.gpsimd.memset(nb5[:], EXP_BIAS)
            zin = [dp.tile([128, j1 - j0], f32, name=f"zin{k}")
                   for k, (j0, j1) in enumerate(SEGS)]
            zout = [dp.tile([128, j1 - j0], f32, addr_space="Shared",
                            name=f"zout{k}")
                    for k, (j0, j1) in enumerate(SEGS)]
            phin = dp.tile([CI, CH], f16, name="phin")
            phout = dp.tile([NCORES, CI, CH], f16, addr_space="Shared",
                            name="phout")
            gin = dp.tile([128, TPC * C], f16, name="gin")
            gout = dp.tile([NCORES, 128, TPC * C], f16, addr_space="Shared",
                           name="gout")

            with tc.tile_pool(name="hand", bufs=1) as hp:
                phi16 = hp.tile([CI, N], f16)
                th16 = hp.tile([CI, CH], f16)
                G16 = hp.tile([128, MT * C], f16)
                G3 = G16[:].rearrange("p (j c) -> p j c", c=C)
                G8 = hp.tile([128, MT * C], f8)
                G83 = G8[:].rearrange("p (j c) -> p j c", c=C)
                R8 = hp.tile([128, MT * C], f8)
                R83 = R8[:].rearrange("p (j c) -> p j c", c=C)
                xgc16 = hp.tile([C, CH], f16)
                outsb = hp.tile([C, CH], f32)
                # (exp-table load is implicit before the first pass-1 exp;
                # it hides behind the phi AllGather landing wait)
                zdeadV = hp.tile([128, CH], f8w)  # dead stores for Z accum
                zdeadP = hp.tile([128, CH], f8w)  # (same dtype as the cache)
                phiown = hp.tile([CI, CH], f16)
                gown = hp.tile([128, TPC * C], f16)
                s8 = hp.tile([128, MT * CH], f8w)
                s83 = s8[:].rearrange("p (j n) -> p j n", n=CH)

                # ==================== PHASE A ====================
                # single merged pool scope: no mid-phase close barrier
                # between the convs and the gate/projection pipeline
                with tc.tile_pool(name="pa", bufs=1) as pa, \
                     tc.tile_pool(name="paps", bufs=2, space="PSUM") as paps:
                    y3Ta = pa.tile([72, C], f16)
                    y3Tb = pa.tile([72, C], f16)
                    # preload the Sigmoid table while input DMAs fly
                    tld0 = pa.tile([128, 1], f32)
                    nc.scalar.activation(tld0[:], nb5[:], AF.Sigmoid)
                    # ramp the PE p-state during the input-DMA wait so conv1
                    # runs at full speed from its first matmul (identity
                    # needs no DMA)
                    ident = pa.tile([C, C], f16)
                    masks.make_identity(nc, ident[:])
                    wmps = paps.tile([C, C], f32, tag="warm", name="wmps",
                                     bufs=1)
                    for _ in range(140):
                        nc.tensor.matmul(wmps[:], ident[:], ident[:],
                                         start=True, stop=True,
                                         skip_group_check=True)

                    # conv-critical DMAs first: HWDGE is one serial queue,
                    # and conv1 must run gapless to keep the PE p-state up
                    xpad = pa.tile([C, 98, 98], f16)
                    w1sb = pa.tile([C, 9 * C], f16)
                    nc.sync.dma_start(xpad[:, 0:50, :], xpad_io[:, 0:50, :])
                    nc.sync.dma_start(w1sb[:], w1_io[:])
                    nc.sync.dma_start(xpad[:, 50:98, :], xpad_io[:, 50:98, :])
                    w2sb = pa.tile([C, 9 * C], f16)
                    nc.sync.dma_start(w2sb[:], w2_io[:])
                    w3sb = pa.tile([C, 9 * C], f16)
                    nc.sync.dma_start(w3sb[:], w3_io[:])
                    twT16 = pa.tile([C, CI], f16)
                    nc.sync.dma_start(twT16[:], twT_io[:])
                    pwT16 = pa.tile([C, CI], f16)
                    nc.sync.dma_start(pwT16[:], pwT_io[:])
                    gwsb = pa.tile([CI, C], f32)
                    nc.sync.dma_start(gwsb[:], gw_io[:])
                    WwTsb = pa.tile([CI, C], f32)
                    nc.sync.dma_start(WwTsb[:], WwT_io[:])
                    krCa = pa.tile([72, CH], f16)
                    nc.sync.dma_start(krCa[:], krC_io[0:72, :])
                    krCb = pa.tile([72, CH], f16)
                    nc.sync.dma_start(krCb[:], krC_io[72:144, :])
                    xchsb = pa.tile([C, CH], f16)
                    nc.sync.dma_start(xchsb[:], xch_io[:])

                    # conv1: 96x96 -> 48x48, stride 2, pad 1, lrelu(0.2)
                    y1p = pa.tile([C, 50, 50], f16)
                    nc.gpsimd.memset(y1p[:], 0.0)
                    for g in range(6):
                        ps1 = paps.tile([C, 8, 48], f32, tag="cv", name="ps1")
                        for t in range(9):
                            dy, dx = t // 3, t % 3
                            nc.tensor.matmul(
                                ps1[:], w1sb[:, t * C:(t + 1) * C],
                                xpad[:, 16 * g + dy: 16 * g + dy + 16: 2,
                                     dx: dx + 96: 2],
                                start=(t == 0), stop=(t == 8))
                        # lrelu(x) = 0.6*x + 0.4*|x| (only one PSUM input
                        # allowed per DVE op; ACT is idle during the convs)
                        ab1 = pa.tile([C, 8 * 48], f32, tag="ab1", name="ab1",
                                      bufs=2)
                        nc.scalar.activation(ab1[:], ps1[:], AF.Abs,
                                             scale=0.4)
                        nc.vector.scalar_tensor_tensor(
                            y1p[:, 1 + 8 * g: 9 + 8 * g, 1:49], ps1[:], 0.6,
                            ab1[:], op0=ALU.mult, op1=ALU.add)

                    # conv2: 48x48 -> 24x24
                    y2p = pa.tile([C, 26, 26], f16)
                    nc.gpsimd.memset(y2p[:], 0.0)
                    for g in range(2):
                        ps2 = paps.tile([C, 12, 24], f32, tag="cv", name="ps2")
                        for t in range(9):
                            dy, dx = t // 3, t % 3
                            nc.tensor.matmul(
                                ps2[:], w2sb[:, t * C:(t + 1) * C],
                                y1p[:, 24 * g + dy: 24 * g + dy + 24: 2,
                                    dx: dx + 48: 2],
                                start=(t == 0), stop=(t == 8))
                        ab2 = pa.tile([C, 12 * 24], f32, tag="ab2", name="ab2",
                                      bufs=2)
                        nc.scalar.activation(ab2[:], ps2[:], AF.Abs,
                                             scale=0.4)
                        nc.vector.scalar_tensor_tensor(
                            y2p[:, 1 + 12 * g: 13 + 12 * g, 1:25], ps2[:], 0.6,
                            ab2[:], op0=ALU.mult, op1=ALU.add)

                    # conv3: 24x24 -> 12x12 (no activation), then PE
                    # transpose into y3T[(row, col), c] halves
                    ps3 = paps.tile([C, 12, 12], f32, tag="cv", name="ps3")
                    for t in range(9):
                        dy, dx = t // 3, t % 3
                        nc.tensor.matmul(
                            ps3[:], w3sb[:, t * C:(t + 1) * C],
                            y2p[:, dy: dy + 24: 2, dx: dx + 24: 2],
                            start=(t == 0), stop=(t == 8))
                    y3f = pa.tile([C, 144], f16)
                    nc.vector.tensor_copy(y3f[:], ps3[:])
                    for hh, y3t in ((0, y3Ta), (1, y3Tb)):
                        pst = paps.tile([72, C], f16, tag="cv", name="pst")
                        nc.tensor.transpose(
                            pst[:], y3f[:, 72 * hh:72 * (hh + 1)], ident[:])
                        nc.vector.tensor_copy(y3t[:], pst[:])

                    # E^T = gw^T WwT [C, C]
                    eps = paps.tile([C, 512], f32, tag="prj", name="eps",
                                    bufs=3)
                    nc.tensor.matmul(eps[:, 0:C], gwsb[:], WwTsb[:],
                                     start=True, stop=True)
                    ET16 = hp.tile([C, C], f16)
                    nc.vector.tensor_copy(ET16[:], eps[:, 0:C])

                    # gate pipeline: all krons first (kron -> sigmoid ->
                    # fp16 gate-mul per sub), then the phi chain (it feeds
                    # the AllGather = the pass-1 critical path), then theta
                    gtc = pa.tile([C, CH], f16)
                    for o0, w in SUBS0:
                        kps = paps.tile([C, 512], f32, tag="prj",
                                        name="kps", bufs=3)
                        nc.tensor.matmul(kps[:, 0:w], y3Ta[:],
                                         krCa[:, o0:o0 + w],
                                         start=True, stop=False)
                        nc.tensor.matmul(kps[:, 0:w], y3Tb[:],
                                         krCb[:, o0:o0 + w],
                                         start=False, stop=True)
                        nc.scalar.activation(gtc[:, o0:o0 + w],
                                             kps[:, 0:w], AF.Sigmoid)
                        nc.vector.tensor_mul(xgc16[:, o0:o0 + w],
                                             gtc[:, o0:o0 + w],
                                             xchsb[:, o0:o0 + w])
                    for o0, w in SUBS0:
                        pps = paps.tile([C, 512], f32, tag="prj",
                                        name="pps", bufs=3)
                        nc.tensor.matmul(pps[0:CI, 0:w], pwT16[:],
                                         xgc16[:, o0:o0 + w],
                                         start=True, stop=True)
                        nc.vector.tensor_copy(phiown[:, o0:o0 + w],
                                              pps[0:CI, 0:w])
                    for o0, w in SUBS0:
                        tps = paps.tile([C, 512], f32, tag="prj",
                                        name="tps", bufs=3)
                        nc.tensor.matmul(tps[0:CI, 0:w], twT16[:],
                                         xgc16[:, o0:o0 + w],
                                         start=True, stop=True)
                        # (GPSIMD cannot read PSUM on HW: copies on DVE)
                        nc.vector.tensor_copy(th16[:, o0:o0 + w],
                                              tps[0:CI, 0:w])

                    # own G^T tiles [128, 9*C] (AllGathered later)
                    gps = paps.tile([128, TPC * C], f32, tag="gps",
                                    name="gps", bufs=1)
                    for u in range(TPC):
                        nc.tensor.matmul(gps[:, u * C:(u + 1) * C],
                                         xgc16[:, u * 128:(u + 1) * 128],
                                         ET16[:], start=True, stop=True)
                    nc.vector.tensor_copy(gown[:], gps[:])

                # ====== PASS 1 with seg-interleaved fp8 PASS 2 + ARs ======
                with tc.tile_pool(name="p1ps", bufs=2, space="PSUM") as p1ps, \
                     tc.tile_pool(name="p2ps", bufs=2, space="PSUM") as p2ps, \
                     tc.tile_pool(name="p2", bufs=1) as p2:
                    # AllGathers emitted inside this scope so no pool-close
                    # barrier or clock alignment gates pass-1 on them.
                    # single-mode convention: ONE DRAM hop stands in for
                    # upload+collective; landing DMAs are modeled in full.
                    if single:
                        nc.sync.dma_start(phout[0, :, :], phiown[:])
                    else:
                        nc.sync.dma_start(phin[:], phiown[:])
                        nc.gpsimd.collective_compute(
                            "AllGather", ALU.bypass, replica_groups=groups,
                            ins=[phin.opt()], outs=[phout.opt()])
                    # land slice r=0 first: it unblocks pass-1 tiles 0-8
                    nc.sync.dma_start(phi16[:, 0:CH], phout[0, :, :])
                    nc.sync.dma_start(
                        phi16[:, CH:].rearrange("c (r n) -> c r n",
                                                r=NCORES - 1),
                        phout[1:, :, :].rearrange("r c n -> c r n"))

                    def emit_G_ag():
                        if single:
                            nc.sync.dma_start(gout[0, :, :], gown[:])
                        else:
                            nc.sync.dma_start(gin[:], gown[:])
                            nc.gpsimd.collective_compute(
                                "AllGather", ALU.bypass,
                                replica_groups=groups,
                                ins=[gin.opt()], outs=[gout.opt()])
                        nc.sync.dma_start(
                            G16[:].rearrange("p (r n) -> p r n", r=NCORES),
                            gout[:].rearrange("r p n -> p r n"))

                    # warm the PE through the AG landing wait with fake
                    # pass-1 tiles read from phiown (already in SBUF)
                    for _ in range(3):
                        wfps = p1ps.tile([128, EIW], f32, tag="fps",
                                         name="fps")
                        for o0 in range(0, EIW, 512):
                            nc.tensor.matmul(wfps[:, o0:o0 + 512],
                                             phiown[:, 0:128],
                                             th16[:, 0:512],
                                             start=True, stop=True)

                    def pass1_instr(i):
                        # one 1536-col exp instruction = 1.33 m-tiles; the
                        # fp8 cache is contiguous so the exp span can cross
                        # m-tile boundaries; Z is per-m-tile off the cache
                        c0 = i * EIW
                        fps = p1ps.tile([128, EIW], f32, tag="fps",
                                        name="fps")
                        edges = {0, EIW}
                        for b in range(512, EIW, 512):
                            edges.add(b)
                        jlo, jhi = c0 // CH, (c0 + EIW - 1) // CH
                        for j in range(jlo, jhi + 1):
                            if c0 < j * CH < c0 + EIW:
                                edges.add(j * CH - c0)
                        edges = sorted(edges)
                        for a, b in zip(edges[:-1], edges[1:]):
                            j = (c0 + a) // CH
                            ta = c0 + a - j * CH
                            nc.tensor.matmul(fps[:, a:b],
                                             phi16[:, j * 128:(j + 1) * 128],
                                             th16[:, ta:ta + (b - a)],
                                             start=True, stop=True)
                        nc.scalar.activation(s8[:, c0:c0 + EIW], fps[:],
                                             AF.Exp, bias=nb5[:], scale=1.0)
                        # Z for every m-tile completed by this instruction
                        for j in range(jlo, jhi + 1):
                            if (j + 1) * CH <= c0 + EIW:
                                k, j0 = _seg_of(j)
                                zcol = zsumk[k][:, j - j0:j - j0 + 1]
                                if _zmode(j) == "dve":
                                    eng, zd = nc.vector, zdeadV
                                else:
                                    eng, zd = nc.gpsimd, zdeadP
                                eng.tensor_scalar(
                                    zd[:], s83[:, j, :], 1.0, 0.0,
                                    op0=ALU.mult, op1=ALU.add,
                                    accum_out=zcol)

                    def allreduce(k):
                        # single-mode convention (as for the AllGathers):
                        # one DRAM hop stands in for upload+collective
                        if single:
                            nc.sync.dma_start(zout[k][:], zsumk[k][:])
                        else:
                            nc.sync.dma_start(zin[k][:], zsumk[k][:])
                            nc.gpsimd.collective_compute(
                                "AllReduce", ALU.add,
                                replica_groups=groups,
                                ins=[zin[k].opt()], outs=[zout[k].opt()])
                        nc.sync.dma_start(zredk[k][:], zout[k][:])

                    def scale_G(k):
                        j0, j1 = SEGS[k]
                        ln = j1 - j0
                        zf = p2.tile([128, 22], f32, tag="zf", name="zf",
                                     bufs=2)
                        # 1/(Z/GSCALE) = GSCALE/Z
                        nc.vector.tensor_scalar(zf[:, 0:ln], zredk[k][:],
                                                1.0 / GSCALE, None,
                                                op0=ALU.mult)
                        rz = p2.tile([128, 22], f32, tag="rz", name="rz",
                                     bufs=2)
                        nc.vector.reciprocal(rz[:, 0:ln], zf[:, 0:ln])
                        rzb = rz[:, 0:ln].unsqueeze(-1).to_broadcast(
                            (128, ln, C))
                        nc.vector.tensor_mul(G3[:, j0:j1, :],
                                             G3[:, j0:j1, :], rzb)
                        nc.vector.tensor_copy(G83[:, j0:j1, :],
                                              G3[:, j0:j1, :])
                        if RESID:
                            # split G into fp8 high + fp8 residual parts
                            rt = p2.tile([128, 22 * C], f16, tag="rt",
                                         name="rt", bufs=2)
                            rt3 = rt[:].rearrange("p (j c) -> p j c", c=C)
                            nc.vector.tensor_sub(rt3[:, 0:ln, :],
                                                 G3[:, j0:j1, :],
                                                 G83[:, j0:j1, :])
                            nc.vector.tensor_copy(R83[:, j0:j1, :],
                                                  rt3[:, 0:ln, :])

                    # pass-2 work units: (k, ci, u); per-segment PSUM
                    # accumulation, DVE adds across segments into outsb
                    units = []
                    for k in range(len(SEGS)):
                        j0, j1 = SEGS[k]
                        for ci in range(len(YSUBS)):
                            for u in range((j1 - j0) // 2):
                                units.append((k, ci, u))
                    emitted = 0
                    cur_ps = {}

                    def emit_unit():
                        nonlocal emitted
                        k, ci, u = units[emitted]
                        j0, j1 = SEGS[k]
                        o0, w = YSUBS[ci]
                        npr = (j1 - j0) // 2
                        jj = j0 + 2 * u
                        if u == 0:
                            cur_ps[ci] = p2ps.tile([C, 512], f32, tag="yps",
                                                   name="yps")
                        yp = cur_ps[ci]
                        nc.tensor.matmul(
                            yp[:, 0:w], G83[:, jj:jj + 2, :],
                            s83[:, jj:jj + 2, o0:o0 + w],
                            start=(u == 0), stop=(not RESID and u == npr - 1),
                            perf_mode=DR, skip_group_check=True)
                        if RESID:
                            nc.tensor.matmul(
                                yp[:, 0:w], R83[:, jj:jj + 2, :],
                                s83[:, jj:jj + 2, o0:o0 + w],
                                start=False, stop=(u == npr - 1),
                                perf_mode=DR, skip_group_check=True)
                        if u == npr - 1:
                            eng = nc.vector
                            osl = outsb[:, o0:o0 + w]
                            if k == 0:
                                eng.tensor_copy(osl, yp[:, 0:w])
                            else:
                                eng.tensor_add(osl, osl, yp[:, 0:w])
                            if k == len(SEGS) - 1:
                                # undo GSCALE pre-scale, add gated residual
                                eng.scalar_tensor_tensor(
                                    osl, osl, 1.0 / GSCALE,
                                    xgc16[:, o0:o0 + w],
                                    op0=ALU.mult, op1=ALU.add)
                                nc.sync.dma_start(out_io[:, o0:o0 + w], osl)
                        emitted += 1

                    # m-tile j's exp completes during exp-instr ei(j)
                    def ei_of(j):
                        return ((j + 1) * CH - 1) // EIW

                    seg_ei = [ei_of(s[1] - 1) for s in SEGS]
                    seg_units = [sum(1 for x in units if x[0] <= k)
                                 for k in range(len(SEGS))]
                    avail = [0]

                    def pump(i):
                        if i == 2:
                            emit_G_ag()
                        for k in range(len(SEGS)):
                            if i == seg_ei[k]:
                                allreduce(k)
                                scale_G(k)
                            if (k < len(SEGS) - 2
                                    and i == seg_ei[k] + MARGINS[k]):
                                # last 2 segs drain after the loop, behind
                                # the PE warm-up (parked units would block
                                # the warm-up and drop the p-state)
                                avail[0] = seg_units[k]
                        budget = BUDGET[0] if i < 30 else BUDGET[1]
                        while emitted < avail[0] and budget > 0:
                            emit_unit()
                            budget -= 1

                    for i in range(EI):
                        pass1_instr(i)
                        pump(i)
                    # keep the PE p-state warm through the final Z-AR wait:
                    # re-run an already-satisfied pair into a scratch bank
                    wps = p2ps.tile([C, 512], f32, tag="yps", name="wps")
                    for _ in range(32):
                        nc.tensor.matmul(wps[:], G83[:, 0:2, :],
                                         s83[:, 0:2, 0:512],
                                         start=True, stop=True, perf_mode=DR,
                                         skip_group_check=True)
                    while emitted < len(units):
                        emit_unit()

    nc.compile()
    return nc


def get_program():
    if "nc" not in _compiled:
        _compiled["nc"] = _build()
    return _compiled["nc"]


def _bilinear_kron():
    """K[(k,j), (R,Cc)] = uv[R,k]*uv[Cc,j] for x8 bilinear upsample 12->96
    (align_corners=False, edge-clamped), split into two 72-row halves."""
    uv = np.zeros((96, 12), np.float64)
    for R in range(96):
        t = (R + 0.5) / 8.0 - 0.5
        k0 = int(np.floor(t))
        fr = t - k0
        for k, wt in ((k0, 1.0 - fr), (k0 + 1, fr)):
            kc = min(max(k, 0), 11)
            uv[R, kc] += wt
    K = np.einsum("Rk,Cj->kjRC", uv, uv).reshape(144, 9216)
    return np.ascontiguousarray(K).astype(np.float16)


def make_in_maps(inputs):
    f16 = np.float16
    x = np.asarray(inputs["x"], np.float32).reshape(C, H, W)
    xflat = np.ascontiguousarray(x.reshape(C, N))
    xpad = np.zeros((C, 98, 98), f16)
    xpad[:, 1:97, 1:97] = x.astype(f16)
    krF = _bilinear_kron()

    def conv_w(w):
        # [o, i, dy, dx] -> [i, (dy dx), o]
        return np.ascontiguousarray(
            np.asarray(w, np.float32).transpose(1, 2, 3, 0).reshape(C, 9 * C)
        ).astype(f16)

    base = {
        "xpad": xpad,
        "w1": conv_w(inputs["d1_w"]),
        "w2": conv_w(inputs["d2_w"]),
        "w3": conv_w(inputs["d3_w"]),
        "twT": np.ascontiguousarray(
            np.asarray(inputs["th_w"], np.float32)[:, :, 0, 0].T).astype(f16),
        "pwT": np.ascontiguousarray(
            np.asarray(inputs["ph_w"], np.float32)[:, :, 0, 0].T).astype(f16),
        "gw": np.ascontiguousarray(
            np.asarray(inputs["g_w"], np.float32)[:, :, 0, 0]),
        "WwT": np.ascontiguousarray(
            np.asarray(inputs["W_w"], np.float32)[:, :, 0, 0].T),
    }
    in_maps = []
    for k in range(NCORES):
        m = dict(base)
        m["xch"] = np.ascontiguousarray(
            xflat[:, k * CH:(k + 1) * CH]).astype(f16)
        m["krC"] = np.ascontiguousarray(krF[:, k * CH:(k + 1) * CH])
        in_maps.append(m)
    return in_maps


def kernel(**inputs):
    from concourse import bass_utils

    nc = get_program()
    in_maps = make_in_maps(inputs)
    res = bass_utils.run_bass_kernel_spmd(nc, in_maps,
                                          core_ids=list(range(NCORES)))
    out = np.concatenate([res.results[k]["out"] for k in range(NCORES)], axis=1)
    return out.reshape(1, C, H, W).astype(np.float32)


# revision 83
# speedup vs baseline: 1.3438x; 1.0183x over previous
"""Trainium2 Bass kernel for AttentiveNonLocalBlock2D (AllGather design).

Sequence-parallel over N=H*W across 8 cores, per the sharding hint's
tensor-parallel scheme: each core computes the gate + projections ONLY for
its own 1152-pixel chunk, then phi [32,1152] and G^T [128,9*64] are
AllGathered (DRAM-staged collectives) to form the full phi [32,9216] /
G [128,72*64] every core needs for its n-slice of the attention.

Per core:
  Phase A (one pool scope, no mid barriers): identity-matmul PE p-state
    warm-up under the input DMAs; 3x stride-2 conv gating unit (fp16 PE,
    lrelu = 0.6x+0.4|x| via ACT Abs + DVE stt); conv3 emits pre-transposed
    y3T halves; bilinear-upsample columns for the OWN chunk only via the
    per-core krC input (y3T^T @ krC) -> sigmoid -> fp16 gate-mul ->
    phi/theta/G^T projections.  The AllGathers + Exp table load launch
    outside the phase-A pools so no close-barrier gates pass-1 on them.
  Pass 1: 54 exp instructions of 1536 cols (1.33 m-tiles each; the fp8
    cache is contiguous so spans may cross m-tile boundaries): PE score
    matmuls fT = phi_tile^T theta_chunk into a 2-buffer PSUM ring, ACT
    exp(f - 2.5) written straight to a float8e5 cache (e5m2: wide range so
    the softmax denominator doesn't lose its tail to subnormal flushing;
    e4m3 loses ~10% of Z's mass).  Z[m] partials via DVE dead-store
    tensor_scalar accumulation over the cache (2x SBUF mode); Z is
    AllReduced in 6 segments.
  Pass 2: per segment G is scaled by GSCALE/Z and split into fp8e4 high +
    residual parts; fp8 DoubleRow matmuls (2 m-tiles/instr, 0.5 cy/col,
    e4 stationary x e5 moving) accumulate into per-segment PSUM banks,
    pace-interleaved between later pass-1 instrs (margins keep not-ready
    units from parking at the PE queue head, which would starve ACT);
    DVE adds across segments, final read-out divides by GSCALE and adds
    the gated residual.  The last two segments drain after the loop behind
    a PE warm-up burst that keeps the p-state up through the final Z
    AllReduce latency.
  Host concatenates the per-core n-chunks.

Single-device build (the TimelineSim timing variant) replaces each
collective with one DRAM-hop DMA (upload straight to the gathered buffer);
landing DMAs are modeled in full.
"""

import sys

if "/opt/trn_rl_repo" not in sys.path:
    sys.path.insert(0, "/opt/trn_rl_repo")

import numpy as np

NCORES = 8
C, CI, H, W = 64, 32, 96, 96
N = H * W            # 9216
CH = N // NCORES     # 1152 pixels per core
MT = N // 128        # 72 m-tiles of 128
TPC = MT // NCORES   # 9 own m-tiles per core
EXP_BIAS = -2.5      # keeps exp(f+bias) <= ~16k < 57344 (e5m2 max) while
                     # minimizing subnormal flushing of tiny softmax terms
GSCALE = 64.0 * float(np.exp(-2.5 + 7.5))
                     # pre-scale so G*GSCALE/Z clears the e4m3 subnormal
                     # floor; tracks EXP_BIAS (Z scales with exp(bias))
SEGS = ((0, 22), (22, 40), (40, 54), (54, 64), (64, 70), (70, 72))
EIW = 1536           # exp-instruction width (cols)
# 52 x 1536-col instrs, then tile-aligned tails (768, 1152, 1152): the last
# two instrs cover exactly tiles 70 / 71 so their Z comes from the ACT f32
# accumulator (saves the DVE round trip on the final Z-AllReduce chain)
INSTRS = tuple([(i * EIW, EIW) for i in range(52)]
               + [(52 * EIW, 768), (70 * CH, CH), (71 * CH, CH)])
MARGINS = (7, 8, 7, 6, 99, 99)  # exp-instrs between AR issue and pass-2
                                # emit; last two segs drain after the loop
BUDGET = (3, 4)      # pass-2 units per exp instr (early, late)
RESID = True         # add an fp8 residual pass for G (extra accuracy)
# n-chunk subtiles for the two PSUM ring halves (bank-boundary aligned)
SUBS0 = ((0, 512), (512, 512), (1024, 128))
SUBS1 = ((0, 384), (384, 512), (896, 256))
YSUBS = ((0, 512), (512, 512), (1024, 128))  # pass-2 output subtiles

_compiled = {}


def _zmode(j):
    """Z accumulation engine per tile: DVE only (the dead-store
    tensor_scalar opcode does not exist on GPSIMD, and ACT's accumulator
    cannot be used because exp instructions span m-tile boundaries)."""
    return "dve"


def _seg_of(j):
    for k, (j0, j1) in enumerate(SEGS):
        if j0 <= j < j1:
            return k, j0
    raise ValueError(j)


def _build(single=False):
    import concourse.bacc as bacc
    import concourse.bass as bass
    import concourse.mybir as mybir
    import concourse.tile as tile
    from concourse import masks

    f16 = mybir.dt.float16
    f32 = mybir.dt.float32
    f8 = mybir.dt.float8e4
    f8w = mybir.dt.float8e5   # exp cache: wide range so tiny softmax terms
                              # aren't flushed (Z would lose ~10% of its mass)
    DR = mybir.MatmulPerfMode.DoubleRow
    AF = mybir.ActivationFunctionType
    ALU = mybir.AluOpType

    nc = bacc.Bacc("TRN2", target_bir_lowering=False, debug=False,
                   num_devices=1 if single else NCORES)

    xpad_io = nc.dram_tensor("xpad", [C, 98, 98], f16, kind="ExternalInput")
    w1_io = nc.dram_tensor("w1", [C, 9 * C], f16, kind="ExternalInput")
    w2_io = nc.dram_tensor("w2", [C, 9 * C], f16, kind="ExternalInput")
    w3_io = nc.dram_tensor("w3", [C, 9 * C], f16, kind="ExternalInput")
    twT_io = nc.dram_tensor("twT", [C, CI], f16, kind="ExternalInput")
    pwT_io = nc.dram_tensor("pwT", [C, CI], f16, kind="ExternalInput")
    gw_io = nc.dram_tensor("gw", [CI, C], f32, kind="ExternalInput")
    WwT_io = nc.dram_tensor("WwT", [CI, C], f32, kind="ExternalInput")
    xch_io = nc.dram_tensor("xch", [C, CH], f16, kind="ExternalInput")
    krC_io = nc.dram_tensor("krC", [144, CH], f16, kind="ExternalInput")
    out_io = nc.dram_tensor("out", [C, CH], f32, kind="ExternalOutput")

    groups = [list(range(NCORES))]

    with tile.TileContext(nc) as tc:
        with tc.tile_pool(name="persist", bufs=1) as pp, \
             tc.tile_pool(name="dram", bufs=1, space="DRAM") as dp:
            # per-segment Z tiles so the AR DMA reads never alias later writes
            zsumk = [pp.tile([128, j1 - j0], f32, name=f"zsum{k}")
                     for k, (j0, j1) in enumerate(SEGS)]
            zredk = [pp.tile([128, j1 - j0], f32, name=f"zred{k}")
                     for k, (j0, j1) in enumerate(SEGS)]
            nb5 = pp.tile([128, 1], f32)
            nc.gpsimd.memset(nb5[:], EXP_BIAS)
            zin = [dp.tile([128, j1 - j0], f32, name=f"zin{k}")
                   for k, (j0, j1) in enumerate(SEGS)]
            zout = [dp.tile([128, j1 - j0], f32, addr_space="Shared",
                            name=f"zout{k}")
                    for k, (j0, j1) in enumerate(SEGS)]
            phin = dp.tile([CI, CH], f16, name="phin")
            phout = dp.tile([NCORES, CI, CH], f16, addr_space="Shared",
                            name="phout")
            gin = dp.tile([128, TPC * C], f16, name="gin")
            gout = dp.tile([NCORES, 128, TPC * C], f16, addr_space="Shared",
                           name="gout")

            with tc.tile_pool(name="hand", bufs=1) as hp:
                phi16 = hp.tile([CI, N], f16)
                th16 = hp.tile([CI, CH], f16)
                G16 = hp.tile([128, MT * C], f16)
                G3 = G16[:].rearrange("p (j c) -> p j c", c=C)
                G8 = hp.tile([128, MT * C], f8)
                G83 = G8[:].rearrange("p (j c) -> p j c", c=C)
                R8 = hp.tile([128, MT * C], f8)
                R83 = R8[:].rearrange("p (j c) -> p j c", c=C)
                xgc16 = hp.tile([C, CH], f16)
                outsb = hp.tile([C, CH], f32)
                # (exp-table load is implicit before the first pass-1 exp;
                # it hides behind the phi AllGather landing wait)
                zdeadV = hp.tile([128, CH], f8w)  # dead stores for Z accum
                zdeadP = hp.tile([128, CH], f8w)  # (same dtype as the cache)
                phiown = hp.tile([CI, CH], f16)
                gown = hp.tile([128, TPC * C], f16)
                s8 = hp.tile([128, MT * CH], f8w)
                s83 = s8[:].rearrange("p (j n) -> p j n", n=CH)

                # ==================== PHASE A ====================
                # single merged pool scope: no mid-phase close barrier
                # between the convs and the gate/projection pipeline
                with tc.tile_pool(name="pa", bufs=1) as pa, \
                     tc.tile_pool(name="paps", bufs=2, space="PSUM") as paps:
                    y3Ta = pa.tile([72, C], f16)
                    y3Tb = pa.tile([72, C], f16)
                    # preload the Sigmoid table while input DMAs fly
                    tld0 = pa.tile([128, 1], f32)
                    nc.scalar.activation(tld0[:], nb5[:], AF.Sigmoid)
                    # ramp the PE p-state during the input-DMA wait so conv1
                    # runs at full speed from its first matmul (identity
                    # needs no DMA)
                    ident = pa.tile([C, C], f16)
                    masks.make_identity(nc, ident[:])
                    wmps = paps.tile([C, C], f32, tag="warm", name="wmps",
                                     bufs=1)
                    for _ in range(140):
                        nc.tensor.matmul(wmps[:], ident[:], ident[:],
                                         start=True, stop=True,
                                         skip_group_check=True)

                    # conv-critical DMAs first: HWDGE is one serial queue,
                    # and conv1 must run gapless to keep the PE p-state up
                    xpad = pa.tile([C, 98, 98], f16)
                    w1sb = pa.tile([C, 9 * C], f16)
                    nc.sync.dma_start(xpad[:, 0:18, :], xpad_io[:, 0:18, :])
                    nc.sync.dma_start(w1sb[:], w1_io[:])
                    nc.sync.dma_start(xpad[:, 18:50, :], xpad_io[:, 18:50, :])
                    nc.sync.dma_start(xpad[:, 50:98, :], xpad_io[:, 50:98, :])
                    w2sb = pa.tile([C, 9 * C], f16)
                    nc.sync.dma_start(w2sb[:], w2_io[:])
                    w3sb = pa.tile([C, 9 * C], f16)
                    nc.sync.dma_start(w3sb[:], w3_io[:])
                    twT16 = pa.tile([C, CI], f16)
                    nc.sync.dma_start(twT16[:], twT_io[:])
                    pwT16 = pa.tile([C, CI], f16)
                    nc.sync.dma_start(pwT16[:], pwT_io[:])
                    gwsb = pa.tile([CI, C], f32)
                    nc.sync.dma_start(gwsb[:], gw_io[:])
                    WwTsb = pa.tile([CI, C], f32)
                    nc.sync.dma_start(WwTsb[:], WwT_io[:])
                    krCa = pa.tile([72, CH], f16)
                    nc.sync.dma_start(krCa[:], krC_io[0:72, :])
                    krCb = pa.tile([72, CH], f16)
                    nc.sync.dma_start(krCb[:], krC_io[72:144, :])
                    xchsb = pa.tile([C, CH], f16)
                    nc.sync.dma_start(xchsb[:], xch_io[:])

                    # conv1: 96x96 -> 48x48, stride 2, pad 1, lrelu(0.2)
                    y1p = pa.tile([C, 50, 50], f16)
                    nc.gpsimd.memset(y1p[:], 0.0)
                    for g in range(6):
                        ps1 = paps.tile([C, 8, 48], f32, tag="cv", name="ps1")
                        for t in range(9):
                            dy, dx = t // 3, t % 3
                            nc.tensor.matmul(
                                ps1[:], w1sb[:, t * C:(t + 1) * C],
                                xpad[:, 16 * g + dy: 16 * g + dy + 16: 2,
                                     dx: dx + 96: 2],
                                start=(t == 0), stop=(t == 8))
                        # lrelu(x) = 0.6*x + 0.4*|x| (only one PSUM input
                        # allowed per DVE op; ACT is idle during the convs)
                        ab1 = pa.tile([C, 8 * 48], f32, tag="ab1", name="ab1",
                                      bufs=2)
                        nc.scalar.activation(ab1[:], ps1[:], AF.Abs,
                                             scale=0.4)
                        nc.vector.scalar_tensor_tensor(
                            y1p[:, 1 + 8 * g: 9 + 8 * g, 1:49], ps1[:], 0.6,
                            ab1[:], op0=ALU.mult, op1=ALU.add)

                    # conv2: 48x48 -> 24x24
                    y2p = pa.tile([C, 26, 26], f16)
                    nc.gpsimd.memset(y2p[:], 0.0)
                    for g in range(2):
                        ps2 = paps.tile([C, 12, 24], f32, tag="cv", name="ps2")
                        for t in range(9):
                            dy, dx = t // 3, t % 3
                            nc.tensor.matmul(
                                ps2[:], w2sb[:, t * C:(t + 1) * C],
                                y1p[:, 24 * g + dy: 24 * g + dy + 24: 2,
                                    dx: dx + 48: 2],
                                start=(t == 0), stop=(t == 8))
                        ab2 = pa.tile([C, 12 * 24], f32, tag="ab2", name="ab2",
                                      bufs=2)
                        nc.scalar.activation(ab2[:], ps2[:], AF.Abs,
                                             scale=0.4)
                        nc.vector.scalar_tensor_tensor(
                            y2p[:, 1 + 12 * g: 13 + 12 * g, 1:25], ps2[:], 0.6,
                            ab2[:], op0=ALU.mult, op1=ALU.add)

                    # conv3: 24x24 -> 12x12 (no activation), then PE
                    # transpose into y3T[(row, col), c] halves
                    ps3 = paps.tile([C, 12, 12], f32, tag="cv", name="ps3")
                    for t in range(9):
                        dy, dx = t // 3, t % 3
                        nc.tensor.matmul(
                            ps3[:], w3sb[:, t * C:(t + 1) * C],
                            y2p[:, dy: dy + 24: 2, dx: dx + 24: 2],
                            start=(t == 0), stop=(t == 8))
                    y3f = pa.tile([C, 144], f16)
                    nc.vector.tensor_copy(y3f[:], ps3[:])
                    for hh, y3t in ((0, y3Ta), (1, y3Tb)):
                        pst = paps.tile([72, C], f16, tag="cv", name="pst")
                        nc.tensor.transpose(
                            pst[:], y3f[:, 72 * hh:72 * (hh + 1)], ident[:])
                        nc.vector.tensor_copy(y3t[:], pst[:])

                    # E^T = gw^T WwT [C, C]
                    eps = paps.tile([C, 512], f32, tag="prj", name="eps",
                                    bufs=3)
                    nc.tensor.matmul(eps[:, 0:C], gwsb[:], WwTsb[:],
                                     start=True, stop=True)
                    ET16 = hp.tile([C, C], f16)
                    nc.vector.tensor_copy(ET16[:], eps[:, 0:C])

                    # gate pipeline: all krons first (kron -> sigmoid ->
                    # fp16 gate-mul per sub), then the phi chain (it feeds
                    # the AllGather = the pass-1 critical path), then theta
                    gtc = pa.tile([C, CH], f16)
                    for o0, w in SUBS0:
                        kps = paps.tile([C, 512], f32, tag="prj",
                                        name="kps", bufs=3)
                        nc.tensor.matmul(kps[:, 0:w], y3Ta[:],
                                         krCa[:, o0:o0 + w],
                                         start=True, stop=False)
                        nc.tensor.matmul(kps[:, 0:w], y3Tb[:],
                                         krCb[:, o0:o0 + w],
                                         start=False, stop=True)
                        nc.scalar.activation(gtc[:, o0:o0 + w],
                                             kps[:, 0:w], AF.Sigmoid)
                        nc.vector.tensor_mul(xgc16[:, o0:o0 + w],
                                             gtc[:, o0:o0 + w],
                                             xchsb[:, o0:o0 + w])
                    # preload the Exp table during the gate pipeline: the read
                    # of gtc pins this after sigmoid0 (it cannot be hoisted
                    # to t=0 where the sigmoid load would evict it again)
                    tld1 = pa.tile([C, 1], f32)
                    nc.scalar.activation(tld1[:], gtc[:, 0:1], AF.Exp)
                    for o0, w in SUBS0:
                        pps = paps.tile([C, 512], f32, tag="prj",
                                        name="pps", bufs=3)
                        nc.tensor.matmul(pps[0:CI, 0:w], pwT16[:],
                                         xgc16[:, o0:o0 + w],
                                         start=True, stop=True)
                        nc.vector.tensor_copy(phiown[:, o0:o0 + w],
                                              pps[0:CI, 0:w])
                    for o0, w in SUBS0:
                        tps = paps.tile([C, 512], f32, tag="prj",
                                        name="tps", bufs=3)
                        nc.tensor.matmul(tps[0:CI, 0:w], twT16[:],
                                         xgc16[:, o0:o0 + w],
                                         start=True, stop=True)
                        # (GPSIMD cannot read PSUM on HW: copies on DVE)
                        nc.vector.tensor_copy(th16[:, o0:o0 + w],
                                              tps[0:CI, 0:w])

                    # own G^T tiles [128, 9*C] (AllGathered later)
                    gps = paps.tile([128, TPC * C], f32, tag="gps",
                                    name="gps", bufs=1)
                    for u in range(TPC):
                        nc.tensor.matmul(gps[:, u * C:(u + 1) * C],
                                         xgc16[:, u * 128:(u + 1) * 128],
                                         ET16[:], start=True, stop=True)
                    nc.vector.tensor_copy(gown[:], gps[:])

                # ====== PASS 1 with seg-interleaved fp8 PASS 2 + ARs ======
                with tc.tile_pool(name="p1ps", bufs=2, space="PSUM") as p1ps, \
                     tc.tile_pool(name="p2ps", bufs=2, space="PSUM") as p2ps, \
                     tc.tile_pool(name="p2", bufs=1) as p2:
                    # AllGathers emitted inside this scope so no pool-close
                    # barrier or clock alignment gates pass-1 on them.
                    # single-mode convention: ONE DRAM hop stands in for
                    # upload+collective; landing DMAs are modeled in full.
                    if single:
                        nc.sync.dma_start(phout[0, :, :], phiown[:])
                    else:
                        nc.sync.dma_start(phin[:], phiown[:])
                        nc.gpsimd.collective_compute(
                            "AllGather", ALU.bypass, replica_groups=groups,
                            ins=[phin.opt()], outs=[phout.opt()])
                    # land slice r=0 first: it unblocks pass-1 tiles 0-8
                    nc.sync.dma_start(phi16[:, 0:CH], phout[0, :, :])
                    nc.sync.dma_start(
                        phi16[:, CH:].rearrange("c (r n) -> c r n",
                                                r=NCORES - 1),
                        phout[1:, :, :].rearrange("r c n -> c r n"))

                    # warm the PE through the AG landing wait with fake
                    # pass-1 tiles read from phiown (already in SBUF)
                    for _ in range(3):
                        wfps = p1ps.tile([128, EIW], f32, tag="fps",
                                         name="fps")
                        for o0 in range(0, EIW, 512):
                            nc.tensor.matmul(wfps[:, o0:o0 + 512],
                                             phiown[:, 0:128],
                                             th16[:, 0:512],
                                             start=True, stop=True)

                    def emit_G_ag():
                        if single:
                            nc.sync.dma_start(gout[0, :, :], gown[:])
                        else:
                            nc.sync.dma_start(gin[:], gown[:])
                            nc.gpsimd.collective_compute(
                                "AllGather", ALU.bypass,
                                replica_groups=groups,
                                ins=[gin.opt()], outs=[gout.opt()])
                        nc.sync.dma_start(
                            G16[:].rearrange("p (r n) -> p r n", r=NCORES),
                            gout[:].rearrange("r p n -> p r n"))

                    def pass1_instr(i):
                        # one exp instruction = up to 1.33 m-tiles; the fp8
                        # cache is contiguous so the exp span can cross
                        # m-tile boundaries; Z is per-m-tile off the cache,
                        # except single-tile-aligned instrs which use the
                        # ACT f32 accumulator directly
                        c0, wd = INSTRS[i]
                        fps = p1ps.tile([128, wd], f32, tag="fps",
                                        name="fps")
                        edges = {0, wd}
                        for b in range(512, wd, 512):
                            edges.add(b)
                        jlo, jhi = c0 // CH, (c0 + wd - 1) // CH
                        for j in range(jlo, jhi + 1):
                            if c0 < j * CH < c0 + wd:
                                edges.add(j * CH - c0)
                        edges = sorted(edges)
                        for a, b in zip(edges[:-1], edges[1:]):
                            j = (c0 + a) // CH
                            ta = c0 + a - j * CH
                            nc.tensor.matmul(fps[:, a:b],
                                             phi16[:, j * 128:(j + 1) * 128],
                                             th16[:, ta:ta + (b - a)],
                                             start=True, stop=True)
                        aligned = (wd == CH and c0 % CH == 0)
                        if aligned:
                            j = c0 // CH
                            k, j0 = _seg_of(j)
                            nc.scalar.activation(
                                s8[:, c0:c0 + wd], fps[:], AF.Exp,
                                bias=nb5[:], scale=1.0,
                                accum_out=zsumk[k][:, j - j0:j - j0 + 1])
                            return
                        nc.scalar.activation(s8[:, c0:c0 + wd], fps[:],
                                             AF.Exp, bias=nb5[:], scale=1.0)
                        # Z for every m-tile completed by this instruction
                        for j in range(jlo, jhi + 1):
                            if (j + 1) * CH <= c0 + wd:
                                k, j0 = _seg_of(j)
                                zcol = zsumk[k][:, j - j0:j - j0 + 1]
                                nc.vector.tensor_scalar(
                                    zdeadV[:], s83[:, j, :], 1.0, 0.0,
                                    op0=ALU.mult, op1=ALU.add,
                                    accum_out=zcol)

                    def allreduce(k):
                        # single-mode convention (as for the AllGathers):
                        # one DRAM hop stands in for upload+collective
                        if single:
                            nc.sync.dma_start(zout[k][:], zsumk[k][:])
                        else:
                            nc.sync.dma_start(zin[k][:], zsumk[k][:])
                            nc.gpsimd.collective_compute(
                                "AllReduce", ALU.add,
                                replica_groups=groups,
                                ins=[zin[k].opt()], outs=[zout[k].opt()])
                        nc.sync.dma_start(zredk[k][:], zout[k][:])

                    def scale_G(k):
                        j0, j1 = SEGS[k]
                        ln = j1 - j0
                        zf = p2.tile([128, 22], f32, tag="zf", name="zf",
                                     bufs=2)
                        # 1/(Z/GSCALE) = GSCALE/Z
                        nc.vector.tensor_scalar(zf[:, 0:ln], zredk[k][:],
                                                1.0 / GSCALE, None,
                                                op0=ALU.mult)
                        rz = p2.tile([128, 22], f32, tag="rz", name="rz",
                                     bufs=2)
                        nc.vector.reciprocal(rz[:, 0:ln], zf[:, 0:ln])
                        rzb = rz[:, 0:ln].unsqueeze(-1).to_broadcast(
                            (128, ln, C))
                        nc.vector.tensor_mul(G3[:, j0:j1, :],
                                             G3[:, j0:j1, :], rzb)
                        nc.vector.tensor_copy(G83[:, j0:j1, :],
                                              G3[:, j0:j1, :])
                        if RESID:
                            # split G into fp8 high + fp8 residual parts
                            rt = p2.tile([128, 22 * C], f16, tag="rt",
                                         name="rt", bufs=2)
                            rt3 = rt[:].rearrange("p (j c) -> p j c", c=C)
                            nc.vector.tensor_sub(rt3[:, 0:ln, :],
                                                 G3[:, j0:j1, :],
                                                 G83[:, j0:j1, :])
                            nc.vector.tensor_copy(R83[:, j0:j1, :],
                                                  rt3[:, 0:ln, :])

                    # pass-2 work units: (k, ci, u); per-segment PSUM
                    # accumulation, DVE adds across segments into outsb
                    units = []
                    for k in range(len(SEGS)):
                        j0, j1 = SEGS[k]
                        for ci in range(len(YSUBS)):
                            for u in range((j1 - j0) // 2):
                                units.append((k, ci, u))
                    emitted = 0
                    cur_ps = {}

                    def emit_unit():
                        nonlocal emitted
                        k, ci, u = units[emitted]
                        j0, j1 = SEGS[k]
                        o0, w = YSUBS[ci]
                        npr = (j1 - j0) // 2
                        jj = j0 + 2 * u
                        if u == 0:
                            cur_ps[ci] = p2ps.tile([C, 512], f32, tag="yps",
                                                   name="yps")
                        yp = cur_ps[ci]
                        nc.tensor.matmul(
                            yp[:, 0:w], G83[:, jj:jj + 2, :],
                            s83[:, jj:jj + 2, o0:o0 + w],
                            start=(u == 0), stop=(not RESID and u == npr - 1),
                            perf_mode=DR, skip_group_check=True)
                        if RESID:
                            nc.tensor.matmul(
                                yp[:, 0:w], R83[:, jj:jj + 2, :],
                                s83[:, jj:jj + 2, o0:o0 + w],
                                start=False, stop=(u == npr - 1),
                                perf_mode=DR, skip_group_check=True)
                        if u == npr - 1:
                            eng = nc.vector
                            osl = outsb[:, o0:o0 + w]
                            if k == 0:
                                eng.tensor_copy(osl, yp[:, 0:w])
                            else:
                                eng.tensor_add(osl, osl, yp[:, 0:w])
                            if k == len(SEGS) - 1:
                                # undo GSCALE pre-scale, add gated residual
                                eng.scalar_tensor_tensor(
                                    osl, osl, 1.0 / GSCALE,
                                    xgc16[:, o0:o0 + w],
                                    op0=ALU.mult, op1=ALU.add)
                                nc.sync.dma_start(out_io[:, o0:o0 + w], osl)
                        emitted += 1

                    # m-tile j's exp completes during exp-instr ei(j)
                    def ei_of(j):
                        end = (j + 1) * CH
                        for i, (c0, wd) in enumerate(INSTRS):
                            if c0 + wd >= end:
                                return i
                        raise ValueError(j)

                    seg_ei = [ei_of(s[1] - 1) for s in SEGS]
                    seg_units = [sum(1 for x in units if x[0] <= k)
                                 for k in range(len(SEGS))]
                    avail = [0]

                    def pump(i):
                        if i == 2:
                            emit_G_ag()
                        for k in range(len(SEGS)):
                            if i == seg_ei[k]:
                                allreduce(k)
                                scale_G(k)
                            if (k < len(SEGS) - 2
                                    and i == seg_ei[k] + MARGINS[k]):
                                # last 2 segs drain after the loop, behind
                                # the PE warm-up (parked units would block
                                # the warm-up and drop the p-state)
                                avail[0] = seg_units[k]
                        budget = BUDGET[0] if i < 30 else BUDGET[1]
                        while emitted < avail[0] and budget > 0:
                            emit_unit()
                            budget -= 1

                    for i in range(len(INSTRS)):
                        pass1_instr(i)
                        pump(i)
                    # keep the PE p-state warm through the final Z-AR wait:
                    # re-run an already-satisfied pair into a scratch bank
                    wps = p2ps.tile([C, 512], f32, tag="yps", name="wps")
                    for _ in range(26):
                        nc.tensor.matmul(wps[:], G83[:, 0:2, :],
                                         s83[:, 0:2, 0:512],
                                         start=True, stop=True, perf_mode=DR,
                                         skip_group_check=True)
                    while emitted < len(units):
                        emit_unit()

    nc.compile()
    return nc


def get_program():
    if "nc" not in _compiled:
        _compiled["nc"] = _build()
    return _compiled["nc"]


def _bilinear_kron():
    """K[(k,j), (R,Cc)] = uv[R,k]*uv[Cc,j] for x8 bilinear upsample 12->96
    (align_corners=False, edge-clamped), split into two 72-row halves."""
    uv = np.zeros((96, 12), np.float64)
    for R in range(96):
        t = (R + 0.5) / 8.0 - 0.5
        k0 = int(np.floor(t))
        fr = t - k0
        for k, wt in ((k0, 1.0 - fr), (k0 + 1, fr)):
            kc = min(max(k, 0), 11)
            uv[R, kc] += wt
    K = np.einsum("Rk,Cj->kjRC", uv, uv).reshape(144, 9216)
    return np.ascontiguousarray(K).astype(np.float16)


def make_in_maps(inputs):
    f16 = np.float16
    x = np.asarray(inputs["x"], np.float32).reshape(C, H, W)
    xflat = np.ascontiguousarray(x.reshape(C, N))
    xpad = np.zeros((C, 98, 98), f16)
    xpad[:, 1:97, 1:97] = x.astype(f16)
    krF = _bilinear_kron()

    def conv_w(w):
        # [o, i, dy, dx] -> [i, (dy dx), o]
        return np.ascontiguousarray(
            np.asarray(w, np.float32).transpose(1, 2, 3, 0).reshape(C, 9 * C)
        ).astype(f16)

    base = {
        "xpad": xpad,
        "w1": conv_w(inputs["d1_w"]),
        "w2": conv_w(inputs["d2_w"]),
        "w3": conv_w(inputs["d3_w"]),
        "twT": np.ascontiguousarray(
            np.asarray(inputs["th_w"], np.float32)[:, :, 0, 0].T).astype(f16),
        "pwT": np.ascontiguousarray(
            np.asarray(inputs["ph_w"], np.float32)[:, :, 0, 0].T).astype(f16),
        "gw": np.ascontiguousarray(
            np.asarray(inputs["g_w"], np.float32)[:, :, 0, 0]),
        "WwT": np.ascontiguousarray(
            np.asarray(inputs["W_w"], np.float32)[:, :, 0, 0].T),
    }
    in_maps = []
    for k in range(NCORES):
        m = dict(base)
        m["xch"] = np.ascontiguousarray(
            xflat[:, k * CH:(k + 1) * CH]).astype(f16)
        m["krC"] = np.ascontiguousarray(krF[:, k * CH:(k + 1) * CH])
        in_maps.append(m)
    return in_maps


def kernel(**inputs):
    from concourse import bass_utils

    nc = get_program()
    in_maps = make_in_maps(inputs)
    res = bass_utils.run_bass_kernel_spmd(nc, in_maps,
                                          core_ids=list(range(NCORES)))
    out = np.concatenate([res.results[k]["out"] for k in range(NCORES)], axis=1)
    return out.reshape(1, C, H, W).astype(np.float32)
